# revision 26
# baseline (speedup 1.0000x reference)
"""Trainium2 Bass kernel for nn_Attention_79998060855419 (sparse_attention).

Pipeline per row i of node1 [131072, 512]:
    x      = concat(node1[i], u_rep)                     # [1024]
    weight = node1[i] @ lin1_w.T + lin1_b                # [1]
    alpha  = sigmoid(weight) + 1                         # in (1, 2)
    h0     = selu(x @ att1_w.T + att1_b)                 # [512]
    h1     = selu(h0 @ att2_w.T + att2_b)                # [128]
    s      = h1 @ att3_w.T + att3_b                      # [1]
    out[i] = entmax_bisect(s, alpha)  over dim of size 1 # [1]

Distribution: pure data-parallel over the neighbor axis — 8 cores x 16384
rows, MLP weights and u_rep replicated (per the sharding hint).

Device-side layout: activations flow transposed (features on partitions,
tokens on the free axis), so node1 is fed as node1.T tiles and every matmul
is weights-stationary.  Host-side prep only reshapes/transposes inputs and
folds biases/constants (u_rep contribution of layer 1, selu scale/offset
into the next layer's weights/biases) — all FLOPs over node1-derived data
run on the NeuronCores.

selu(t) = SC*relu(t) + SC*A*(exp(min(t,0)) - 1) is computed as
    e'  = exp(t + ln A)            (ScalarE, bias-folded)
    r'  = max(t, -b) + b           (VectorE tensor_scalar, = relu(t+b) pre-bias)
    nc_ = min(e', A) + r'          (VectorE scalar_tensor_tensor)
with the affine map  selu = SC*nc_ - SC*A  folded into the next layer's
weights/bias on the host.

entmax_bisect with d=1 degenerates: tau_hi == tau_lo == z - 1, dm0 == 0, so
every bisection iteration computes p = clip(z - (z-1), 0)^(1/(alpha-1)) and
the result is p / sum(p) = p / p.  The kernel computes exactly that
(z = s*(alpha-1), t = z - (z-1), p = exp(ln(t) * (1/(alpha-1))), out = p/p),
which is bit-identical to running the 50 fixed-point iterations.
"""

import math

import numpy as np

import concourse.bacc as bacc
import concourse.mybir as mybir
import concourse.tile as tile
from concourse.bass_utils import run_bass_kernel_spmd

N = 131072
D = 512
N_CORES = 8
TPC = N // N_CORES          # tokens per core = 16384
BLK = 512                   # tokens per block
NBLK = TPC // BLK           # 32 blocks per core

SC = 1.0507009873554804934193349852946   # selu scale
A = 1.6732632423543772848170429916717    # selu alpha
LN_A = math.log(A)

F32 = mybir.dt.float32
F32R = mybir.dt.float32r     # fp32 single-pass PE mode (full-rate streaming)
FP8 = mybir.dt.float8e4      # e4m3
BF16 = mybir.dt.bfloat16
USE_FP8 = True               # layer-1 / lin1 matmuls in fp8 with DoubleRow
AF = mybir.ActivationFunctionType
ALU = mybir.AluOpType

_CACHE = {}


def _build(nblk=NBLK, cfg=None):
    base = dict(n1b=3, eb=3, rb=3, h0b=8, h1b=2, stb=4,
                ps1b=5, ps2b=1, pssb=1, pswb=1)
    base.update(cfg or {})
    cfg = base
    key = ("nc", nblk, tuple(sorted(cfg.items())))
    if key in _CACHE:
        return _CACHE[key]

    nc = bacc.Bacc("TRN2", target_bir_lowering=False, debug=False,
                   num_devices=N_CORES)

    # Per-core inputs (shard of node1.T + replicated, host-folded weights).
    DT1 = FP8 if USE_FP8 else F32R
    n1t_d = nc.dram_tensor("n1t", [D, TPC], DT1, kind="ExternalInput")
    w1at_d = nc.dram_tensor("w1at", [D, D], DT1, kind="ExternalInput")
    lin1t_d = nc.dram_tensor("lin1t", [D, 16], DT1, kind="ExternalInput")
    w2te_d = nc.dram_tensor("w2te", [D, 128], BF16, kind="ExternalInput")
    w3te_d = nc.dram_tensor("w3te", [128, 1], BF16, kind="ExternalInput")
    be1_d = nc.dram_tensor("be1", [D, 1], F32, kind="ExternalInput")
    bnr1_d = nc.dram_tensor("bnr1", [D, 1], F32, kind="ExternalInput")
    bpr1_d = nc.dram_tensor("bpr1", [D, 1], F32, kind="ExternalInput")
    be2_d = nc.dram_tensor("be2", [128, 1], F32, kind="ExternalInput")
    bnr2_d = nc.dram_tensor("bnr2", [128, 1], F32, kind="ExternalInput")
    bpr2_d = nc.dram_tensor("bpr2", [128, 1], F32, kind="ExternalInput")
    # per-partition broadcasts of scalar consts for the entmax chain
    b3bc_d = nc.dram_tensor("b3bc", [NBLK, 1], F32, kind="ExternalInput")
    lbbc_d = nc.dram_tensor("lbbc", [NBLK, 1], F32, kind="ExternalInput")
    out_d = nc.dram_tensor("out", [TPC, 1], F32, kind="ExternalOutput")

    with tile.TileContext(nc) as tc:
        with (
            tc.tile_pool(name="wp", bufs=1) as wp,
            tc.tile_pool(name="n1p", bufs=cfg["n1b"]) as n1p,
            tc.tile_pool(name="ep", bufs=cfg["eb"]) as ep,
            tc.tile_pool(name="rp", bufs=cfg["rb"]) as rp,
            tc.tile_pool(name="h0p", bufs=cfg["h0b"]) as h0p,
            tc.tile_pool(name="h1p", bufs=cfg["h1b"]) as h1p,
            tc.tile_pool(name="swp", bufs=1) as swp,
            tc.tile_pool(name="stp", bufs=cfg["stb"]) as stp,
            tc.tile_pool(name="chp", bufs=1) as chp,
            tc.tile_pool(name="ps1p", bufs=cfg["ps1b"], space="PSUM") as ps1p,
            tc.tile_pool(name="ps2p", bufs=cfg["ps2b"], space="PSUM") as ps2p,
            tc.tile_pool(name="pssp", bufs=cfg["pssb"], space="PSUM") as pssp,
            tc.tile_pool(name="pswp", bufs=cfg["pswb"], space="PSUM") as pswp,
        ):
            # ---- replicated weights / biases to SBUF (k-chunks along free) --
            w1a = wp.tile([128, 4, D], DT1, tag="w1a")
            nc.sync.dma_start(
                w1a[:], w1at_d[:].rearrange("(k p) m -> p k m", p=128))
            lin1 = wp.tile([128, 4, 16], DT1, tag="lin1")
            nc.sync.dma_start(
                lin1[:], lin1t_d[:].rearrange("(k p) o -> p k o", p=128))
            w2 = wp.tile([128, 4 * 128], BF16, tag="w2")
            nc.sync.dma_start(
                w2[:], w2te_d[:].rearrange("(k p) m -> p k m", p=128))
            w3 = wp.tile([128, 1], BF16, tag="w3")
            nc.sync.dma_start(w3[:], w3te_d[:])
            be1 = wp.tile([128, 4], F32, tag="be1")
            nc.sync.dma_start(
                be1[:], be1_d[:].rearrange("(k p) o -> p k o", p=128))
            bnr1 = wp.tile([128, 4], F32, tag="bnr1")
            nc.sync.dma_start(
                bnr1[:], bnr1_d[:].rearrange("(k p) o -> p k o", p=128))
            bpr1 = wp.tile([128, 4], F32, tag="bpr1")
            nc.sync.dma_start(
                bpr1[:], bpr1_d[:].rearrange("(k p) o -> p k o", p=128))
            be2 = wp.tile([128, 1], F32, tag="be2")
            nc.sync.dma_start(be2[:], be2_d[:])
            bnr2 = wp.tile([128, 1], F32, tag="bnr2")
            nc.sync.dma_start(bnr2[:], bnr2_d[:])
            bpr2 = wp.tile([128, 1], F32, tag="bpr2")
            nc.sync.dma_start(bpr2[:], bpr2_d[:])
            b3bc = wp.tile([NBLK, 1], F32, tag="b3bc")
            nc.sync.dma_start(b3bc[:], b3bc_d[:])
            lbbc = wp.tile([NBLK, 1], F32, tag="lbbc")
            nc.sync.dma_start(lbbc[:], lbbc_d[:])

            # s / w staging: row b holds tokens [b*BLK, (b+1)*BLK) of the shard
            s32 = swp.tile([NBLK, BLK], F32, tag="s32")
            w32 = swp.tile([NBLK, BLK], F32, tag="w32")

            # Software-pipelined emission: PE executes its queue in order, so
            # layer-2 of block b-1 and layer-3 of block b-2 are emitted under
            # layer-1 of block b — their selu inputs are ready by then and PE
            # never stalls on the ACT/DVE selu chains.
            def emit_l1(b):
                n1 = n1p.tile([128, 4, BLK], DT1, tag="n1")
                nc.sync.dma_start(
                    n1[:],
                    n1t_d[:, b * BLK:(b + 1) * BLK]
                    .rearrange("(k p) t -> p k t", p=128))
                h0s = []
                for m in range(4):
                    ps1 = ps1p.tile([128, BLK], F32, tag="ps1")
                    if USE_FP8:
                        for j in range(2):
                            nc.tensor.matmul(
                                ps1[:],
                                w1a[:, 2 * j:2 * j + 2,
                                    m * 128:(m + 1) * 128],
                                n1[:, 2 * j:2 * j + 2, :],
                                perf_mode=mybir.MatmulPerfMode.DoubleRow,
                                start=(j == 0), stop=(j == 1))
                    else:
                        for k in range(4):
                            nc.tensor.matmul(
                                ps1[:],
                                w1a[:, k, m * 128:(m + 1) * 128],
                                n1[:, k, :],
                                start=(k == 0), stop=(k == 3))
                    e = ep.tile([128, BLK], BF16, tag="e")
                    nc.scalar.activation(e[:], ps1[:], AF.Exp,
                                         bias=be1[:, m:m + 1])
                    q = rp.tile([128, BLK], BF16, tag="q")
                    nc.vector.tensor_scalar(q[:], e[:],
                                            A, bpr1[:, m:m + 1],
                                            ALU.min, ALU.add)
                    h0 = h0p.tile([128, BLK], BF16, tag="h0")
                    nc.vector.scalar_tensor_tensor(h0[:], ps1[:],
                                                   bnr1[:, m:m + 1], q[:],
                                                   ALU.max, ALU.add)
                    h0s.append(h0)
                psw = pswp.tile([1, BLK], F32, tag="psw")
                if USE_FP8:
                    for j in range(2):
                        nc.tensor.matmul(psw[:],
                                         lin1[:, 2 * j:2 * j + 2, 0:1],
                                         n1[:, 2 * j:2 * j + 2, :],
                                         perf_mode=mybir.MatmulPerfMode.DoubleRow,
                                         start=(j == 0), stop=(j == 1))
                else:
                    for k in range(4):
                        nc.tensor.matmul(psw[:], lin1[:, k, 0:1],
                                         n1[:, k, :],
                                         start=(k == 0), stop=(k == 3))
                # raw w (lin1_b and negation folded into the chain's exp)
                wst = stp.tile([1, BLK], F32, tag="wst")
                nc.scalar.copy(wst[:], psw[:])
                nc.sync.dma_start(w32[b:b + 1, :], wst[:])
                return h0s

            def emit_l2(b, h0s):
                ps2 = ps2p.tile([128, BLK], F32, tag="ps2")
                for k in range(4):
                    nc.tensor.matmul(ps2[:], w2[:, k * 128:(k + 1) * 128],
                                     h0s[k][:], start=(k == 0), stop=(k == 3))
                e2 = ep.tile([128, BLK], BF16, tag="e2")
                nc.scalar.activation(e2[:], ps2[:], AF.Exp, bias=be2[:])
                q2 = rp.tile([128, BLK], BF16, tag="q2")
                nc.vector.tensor_scalar(q2[:], e2[:], A, bpr2[:],
                                        ALU.min, ALU.add)
                h1 = h1p.tile([128, BLK], BF16, tag="h1")
                nc.vector.scalar_tensor_tensor(h1[:], ps2[:], bnr2[:], q2[:],
                                               ALU.max, ALU.add)
                return h1

            def emit_l3(b, h1):
                pss = pssp.tile([1, BLK], F32, tag="pss")
                nc.tensor.matmul(pss[:], w3[:], h1[:], start=True, stop=True)
                sst = stp.tile([1, BLK], F32, tag="sst")
                nc.scalar.copy(sst[:], pss[:])
                nc.sync.dma_start(s32[b:b + 1, :], sst[:])

            stage = []   # [(b, h0s or h1, depth)]
            pend_l2 = None   # (b, h0s)
            pend_l3 = None   # (b, h1)
            for b in range(nblk):
                h0s = emit_l1(b)
                if pend_l3 is not None:
                    emit_l3(*pend_l3)
                    pend_l3 = None
                if pend_l2 is not None:
                    pb, ph0s = pend_l2
                    pend_l3 = (pb, emit_l2(pb, ph0s))
                pend_l2 = (b, h0s)
            if pend_l3 is not None:
                emit_l3(*pend_l3)
            if pend_l2 is not None:
                pb, ph0s = pend_l2
                emit_l3(pb, emit_l2(pb, ph0s))

            # ---- entmax_bisect (dim of size 1) over all tokens ------------
            # weight = w32 + lin1_b;  alpha - 1 = sigmoid(weight) = 1/d
            t1 = chp.tile([NBLK, BLK], F32, tag="t1")
            nc.scalar.activation(t1[:], w32[:], AF.Exp,
                                 bias=lbbc[:], scale=-1.0)      # e^{-weight}
            dd = chp.tile([NBLK, BLK], F32, tag="dd")
            nc.vector.tensor_scalar_add(dd[:], t1[:], 1.0)      # 1/(alpha-1)
            rd = chp.tile([NBLK, BLK], F32, tag="rd")
            nc.vector.reciprocal(rd[:], dd[:])                  # alpha-1
            z = chp.tile([NBLK, BLK], F32, tag="z")
            nc.vector.scalar_tensor_tensor(z[:], s32[:], b3bc[:], rd[:],
                                           ALU.add, ALU.mult)
            zm1 = chp.tile([NBLK, BLK], F32, tag="zm1")
            nc.vector.tensor_scalar_sub(zm1[:], z[:], 1.0)      # tau
            tq = chp.tile([NBLK, BLK], F32, tag="tq")
            nc.vector.tensor_tensor(tq[:], z[:], zm1[:], ALU.subtract)
            lq = chp.tile([NBLK, BLK], F32, tag="lq")
            nc.scalar.activation(lq[:], tq[:], AF.Ln)
            le = chp.tile([NBLK, BLK], F32, tag="le")
            nc.vector.tensor_tensor(le[:], lq[:], dd[:], ALU.mult)
            p = chp.tile([NBLK, BLK], F32, tag="p")
            nc.scalar.activation(p[:], le[:], AF.Exp)
            rp = chp.tile([NBLK, BLK], F32, tag="rp")
            nc.vector.reciprocal(rp[:], p[:])
            res = chp.tile([NBLK, BLK], F32, tag="res")
            nc.vector.tensor_tensor(res[:], p[:], rp[:], ALU.mult)

            nc.sync.dma_start(
                out_d[:].rearrange("(q t) o -> q (t o)", q=NBLK), res[:])

    nc.compile()
    _CACHE[key] = nc
    return nc


def _prep_host(node1, u_rep, att1_w, att1_b, att2_w, att2_b, att3_w, att3_b,
               lin1_w, lin1_b):
    import ml_dtypes
    f32 = np.float32
    node1 = np.asarray(node1, f32)
    att1_w = np.asarray(att1_w, f32)
    att2_w = np.asarray(att2_w, f32)
    att3_w = np.asarray(att3_w, f32)
    lin1_w = np.asarray(lin1_w, f32)
    u_rep = np.asarray(u_rep, f32)
    C = np.float32(SC * A)

    # layer 1: u_rep's contribution + att1_b folded into per-feature bias
    u_bias = (att1_w[:, D:] @ u_rep[0] + np.asarray(att1_b, f32)).astype(f32)
    dt1 = ml_dtypes.float8_e4m3 if USE_FP8 else f32
    w1at = np.ascontiguousarray(att1_w[:, :D].T).astype(dt1)  # [D, D]
    be1 = (u_bias + np.float32(LN_A)).reshape(D, 1)
    bnr1 = (-u_bias).reshape(D, 1)
    bpr1 = u_bias.reshape(D, 1).copy()

    # selu affine (h = SC*nc - C) folded into layer 2
    w2te = np.ascontiguousarray(
        (SC * att2_w.T).astype(ml_dtypes.bfloat16))           # [D, 128] bf16
    b2_eff = (np.asarray(att2_b, f32) - C * att2_w.sum(axis=1)).astype(f32)
    be2 = (b2_eff + np.float32(LN_A)).reshape(128, 1)
    bnr2 = (-b2_eff).reshape(128, 1)
    bpr2 = b2_eff.reshape(128, 1).copy()

    # selu affine folded into layer 3
    w3te = np.ascontiguousarray(
        (SC * att3_w.T).astype(ml_dtypes.bfloat16))           # [128, 1] bf16
    b3_eff = np.float32(np.asarray(att3_b, f32)[0] - C * att3_w.sum())

    lin1t = np.zeros((D, 16), f32)
    lin1t[:, 0] = lin1_w[0]
    lin1t = lin1t.astype(dt1)                                 # [D, 16] padded
    b3bc = np.full((NBLK, 1), b3_eff, f32)
    lbbc = np.full((NBLK, 1), -np.float32(np.asarray(lin1_b, f32)[0]), f32)

    shared = dict(w1at=w1at, lin1t=lin1t, w2te=w2te, w3te=w3te, b3bc=b3bc,
                  lbbc=lbbc,
                  be1=np.ascontiguousarray(be1), bnr1=np.ascontiguousarray(bnr1),
                  bpr1=np.ascontiguousarray(bpr1), be2=np.ascontiguousarray(be2),
                  bnr2=np.ascontiguousarray(bnr2), bpr2=np.ascontiguousarray(bpr2))
    in_maps = []
    for c in range(N_CORES):
        m = dict(shared)
        m["n1t"] = np.ascontiguousarray(
            node1[c * TPC:(c + 1) * TPC, :].T).astype(dt1)
        in_maps.append(m)
    return in_maps


def kernel(node1, u_rep, att1_w, att1_b, att2_w, att2_b, att3_w, att3_b,
           lin1_w, lin1_b, num_neighs=None, **_unused):
    nc = _build()
    in_maps = _prep_host(node1, u_rep, att1_w, att1_b, att2_w, att2_b,
                         att3_w, att3_b, lin1_w, lin1_b)
    res = run_bass_kernel_spmd(nc, in_maps, core_ids=list(range(N_CORES)))
    out = np.concatenate([res.results[c]["out"] for c in range(N_CORES)],
                         axis=0)
    return out.astype(np.float32)


# revision 32
# speedup vs baseline: 6553.9714x; 6553.9714x over previous
"""Trainium2 Bass kernel for nn_Attention_79998060855419 (sparse_attention).

Reference pipeline per row i of node1 [131072, 512]:
    x      = concat(node1[i], u_rep)                     # [1024]
    weight = node1[i] @ lin1_w.T + lin1_b                # [1]
    alpha  = sigmoid(weight) + 1                         # in (1, 2)
    h0     = selu(x @ att1_w.T + att1_b)                 # [512]
    h1     = selu(h0 @ att2_w.T + att2_b)                # [128]
    s      = h1 @ att3_w.T + att3_b                      # [1]
    out[i] = entmax_bisect(s, alpha)  over dim of size 1 # [1]

Distribution: pure data-parallel over the neighbor axis — 8 cores x 16384
rows; the tiny MLP weights and u_rep are replicated (per the sharding hint).
No collectives are needed; each core computes its shard's output.

Device-side dataflow (per core, 32 blocks of 512 tokens):
  - Activations flow transposed (features on partitions, tokens on the free
    axis): node1 is fed as node1.T tiles and every matmul is
    weights-stationary, so the M=1 reductions (lin1, att3) are cheap N=512
    matmuls and the entmax stage works on dense [32, 512] tiles.
  - Host prep only reshapes/transposes inputs and folds biases and the selu
    affine constants into downstream weights — all FLOPs over node1-derived
    data run on the NeuronCores.
  - Layer 1 (512x512) and lin1 run on the TensorEngine in fp8(e4m3) with
    perf_mode=DoubleRow (contraction packed in K-pairs, FD=512); layers 2/3
    run in bf16.  The final entmax normalization makes the output invariant
    to these precision choices (p/p == 1.0 bit-exactly either way).
  - selu(t), t = x + u (u = per-feature bias), is computed as
        e' = exp(x + (u + ln A))        (ScalarE, PSUM -> SBUF bf16)
        q  = min(e', A) + u             (VectorE tensor_scalar, bf16 4x)
        nc = max(x, -u) + q             (VectorE scalar_tensor_tensor)
    which equals selu(t)/SC + A; the affine map selu = SC*nc - SC*A is
    folded into the next layer's weights/bias on the host.
  - entmax_bisect with last-dim size 1 degenerates: tau_hi == tau_lo == z-1
    and dm0 == 0, so all 50 bisection iterations compute
    p = clip(z - (z-1), 0)^(1/(alpha-1)) and return p / sum(p) = p / p.
    The kernel computes exactly that (z = s*(alpha-1), t = z - (z-1),
    p = exp(ln(t) * 1/(alpha-1)), out = p * recip(p)) — numerically
    identical to running the fixed-point loop (the result is exactly 1.0
    for every finite positive p, on device and in the reference alike).
"""

import math

import numpy as np

import concourse.bacc as bacc
import concourse.mybir as mybir
import concourse.tile as tile
from concourse.bass_utils import run_bass_kernel_spmd

N = 131072
D = 512
N_CORES = 8
TPC = N // N_CORES          # tokens per core = 16384
BLK = 512                   # tokens per block
NBLK = TPC // BLK           # 32 blocks per core

SC = 1.0507009873554804934193349852946   # selu scale
A = 1.6732632423543772848170429916717    # selu alpha
LN_A = math.log(A)

F32 = mybir.dt.float32
FP8 = mybir.dt.float8e4      # e4m3
BF16 = mybir.dt.bfloat16
AF = mybir.ActivationFunctionType
ALU = mybir.AluOpType
DR = mybir.MatmulPerfMode.DoubleRow

_CACHE = {}


def _build(nblk=NBLK, debug_sw=False):
    key = ("nc", nblk, debug_sw)
    if key in _CACHE:
        return _CACHE[key]

    nc = bacc.Bacc("TRN2", target_bir_lowering=False, debug=False,
                   num_devices=N_CORES)

    # Per-core inputs (shard of node1.T + replicated, host-folded weights).
    n1t_d = nc.dram_tensor("n1t", [D, TPC], FP8, kind="ExternalInput")
    w1at_d = nc.dram_tensor("w1at", [D, D], FP8, kind="ExternalInput")
    lin1t_d = nc.dram_tensor("lin1t", [D, 16], FP8, kind="ExternalInput")
    w2te_d = nc.dram_tensor("w2te", [D, 128], BF16, kind="ExternalInput")
    w3te_d = nc.dram_tensor("w3te", [128, 1], BF16, kind="ExternalInput")
    be1_d = nc.dram_tensor("be1", [D, 1], F32, kind="ExternalInput")
    bnr1_d = nc.dram_tensor("bnr1", [D, 1], F32, kind="ExternalInput")
    bpr1_d = nc.dram_tensor("bpr1", [D, 1], F32, kind="ExternalInput")
    be2_d = nc.dram_tensor("be2", [128, 1], F32, kind="ExternalInput")
    bnr2_d = nc.dram_tensor("bnr2", [128, 1], F32, kind="ExternalInput")
    bpr2_d = nc.dram_tensor("bpr2", [128, 1], F32, kind="ExternalInput")
    # per-partition broadcasts of scalar consts for the entmax chain
    b3bc_d = nc.dram_tensor("b3bc", [NBLK, 1], F32, kind="ExternalInput")
    lbbc_d = nc.dram_tensor("lbbc", [NBLK, 1], F32, kind="ExternalInput")
    out_d = nc.dram_tensor("out", [TPC, 1], F32, kind="ExternalOutput")
    dbg_d = (nc.dram_tensor("dbg", [2 * NBLK, BLK], F32, kind="ExternalOutput")
             if debug_sw else None)

    with tile.TileContext(nc) as tc:
        with (
            tc.tile_pool(name="wp", bufs=1) as wp,
            tc.tile_pool(name="n1p", bufs=3) as n1p,
            tc.tile_pool(name="ep", bufs=3) as ep,
            tc.tile_pool(name="rp", bufs=3) as rp,
            tc.tile_pool(name="h0p", bufs=8) as h0p,
            tc.tile_pool(name="h1p", bufs=2) as h1p,
            tc.tile_pool(name="swp", bufs=1) as swp,
            tc.tile_pool(name="stp", bufs=4) as stp,
            tc.tile_pool(name="chp", bufs=1) as chp,
            tc.tile_pool(name="ps1p", bufs=5, space="PSUM") as ps1p,
            tc.tile_pool(name="ps2p", bufs=1, space="PSUM") as ps2p,
            tc.tile_pool(name="pssp", bufs=1, space="PSUM") as pssp,
            tc.tile_pool(name="pswp", bufs=1, space="PSUM") as pswp,
        ):
            # ---- replicated weights / biases in SBUF ----------------------
            w1a = wp.tile([128, 4, D], FP8, tag="w1a")
            nc.sync.dma_start(
                w1a[:], w1at_d[:].rearrange("(k p) m -> p k m", p=128))
            lin1 = wp.tile([128, 4, 16], FP8, tag="lin1")
            nc.sync.dma_start(
                lin1[:], lin1t_d[:].rearrange("(k p) o -> p k o", p=128))
            w2 = wp.tile([128, 4 * 128], BF16, tag="w2")
            nc.sync.dma_start(
                w2[:], w2te_d[:].rearrange("(k p) m -> p k m", p=128))
            w3 = wp.tile([128, 1], BF16, tag="w3")
            nc.sync.dma_start(w3[:], w3te_d[:])
            be1 = wp.tile([128, 4], F32, tag="be1")
            nc.sync.dma_start(
                be1[:], be1_d[:].rearrange("(k p) o -> p k o", p=128))
            bnr1 = wp.tile([128, 4], F32, tag="bnr1")
            nc.sync.dma_start(
                bnr1[:], bnr1_d[:].rearrange("(k p) o -> p k o", p=128))
            bpr1 = wp.tile([128, 4], F32, tag="bpr1")
            nc.sync.dma_start(
                bpr1[:], bpr1_d[:].rearrange("(k p) o -> p k o", p=128))
            be2 = wp.tile([128, 1], F32, tag="be2")
            nc.sync.dma_start(be2[:], be2_d[:])
            bnr2 = wp.tile([128, 1], F32, tag="bnr2")
            nc.sync.dma_start(bnr2[:], bnr2_d[:])
            bpr2 = wp.tile([128, 1], F32, tag="bpr2")
            nc.sync.dma_start(bpr2[:], bpr2_d[:])
            b3bc = wp.tile([NBLK, 1], F32, tag="b3bc")
            nc.sync.dma_start(b3bc[:], b3bc_d[:])
            lbbc = wp.tile([NBLK, 1], F32, tag="lbbc")
            nc.sync.dma_start(lbbc[:], lbbc_d[:])

            # s / w staging: row b holds tokens [b*BLK, (b+1)*BLK) of the shard
            s32 = swp.tile([NBLK, BLK], F32, tag="s32")
            w32 = swp.tile([NBLK, BLK], F32, tag="w32")

            # ---- per-block emitters (software-pipelined below) ------------
            def emit_l1(b):
                n1 = n1p.tile([128, 4, BLK], FP8, tag="n1")
                nc.sync.dma_start(
                    n1[:],
                    n1t_d[:, b * BLK:(b + 1) * BLK]
                    .rearrange("(k p) t -> p k t", p=128))
                h0s = []
                for m in range(4):
                    ps1 = ps1p.tile([128, BLK], F32, tag="ps1")
                    for j in range(2):       # DoubleRow K pairs (K=2x128)
                        nc.tensor.matmul(
                            ps1[:],
                            w1a[:, 2 * j:2 * j + 2, m * 128:(m + 1) * 128],
                            n1[:, 2 * j:2 * j + 2, :],
                            perf_mode=DR, start=(j == 0), stop=(j == 1))
                    e = ep.tile([128, BLK], BF16, tag="e")
                    nc.scalar.activation(e[:], ps1[:], AF.Exp,
                                         bias=be1[:, m:m + 1])
                    q = rp.tile([128, BLK], BF16, tag="q")
                    nc.vector.tensor_scalar(q[:], e[:],
                                            A, bpr1[:, m:m + 1],
                                            ALU.min, ALU.add)
                    h0 = h0p.tile([128, BLK], BF16, tag="h0")
                    nc.vector.scalar_tensor_tensor(h0[:], ps1[:],
                                                   bnr1[:, m:m + 1], q[:],
                                                   ALU.max, ALU.add)
                    h0s.append(h0)
                psw = pswp.tile([1, BLK], F32, tag="psw")
                for j in range(2):
                    nc.tensor.matmul(psw[:], lin1[:, 2 * j:2 * j + 2, 0:1],
                                     n1[:, 2 * j:2 * j + 2, :],
                                     perf_mode=DR,
                                     start=(j == 0), stop=(j == 1))
                # raw w (lin1_b and negation folded into the chain's exp)
                wst = stp.tile([1, BLK], F32, tag="wst")
                nc.scalar.copy(wst[:], psw[:])
                nc.sync.dma_start(w32[b:b + 1, :], wst[:])
                return h0s

            def emit_l2(b, h0s):
                ps2 = ps2p.tile([128, BLK], F32, tag="ps2")
                for k in range(4):
                    nc.tensor.matmul(ps2[:], w2[:, k * 128:(k + 1) * 128],
                                     h0s[k][:], start=(k == 0), stop=(k == 3))
                e2 = ep.tile([128, BLK], BF16, tag="e2")
                nc.scalar.activation(e2[:], ps2[:], AF.Exp, bias=be2[:])
                q2 = rp.tile([128, BLK], BF16, tag="q2")
                nc.vector.tensor_scalar(q2[:], e2[:], A, bpr2[:],
                                        ALU.min, ALU.add)
                h1 = h1p.tile([128, BLK], BF16, tag="h1")
                nc.vector.scalar_tensor_tensor(h1[:], ps2[:], bnr2[:], q2[:],
                                               ALU.max, ALU.add)
                return h1

            def emit_l3(b, h1):
                pss = pssp.tile([1, BLK], F32, tag="pss")
                nc.tensor.matmul(pss[:], w3[:], h1[:], start=True, stop=True)
                sst = stp.tile([1, BLK], F32, tag="sst")
                nc.scalar.copy(sst[:], pss[:])
                nc.sync.dma_start(s32[b:b + 1, :], sst[:])

            # PE executes its queue in order: L2 of block b-1 and L3 of block
            # b-2 are emitted under L1 of block b, so the PE never waits on
            # the ACT/DVE selu chains.
            pend_l2 = None
            pend_l3 = None
            for b in range(nblk):
                h0s = emit_l1(b)
                if pend_l3 is not None:
                    emit_l3(*pend_l3)
                    pend_l3 = None
                if pend_l2 is not None:
                    pb, ph0s = pend_l2
                    pend_l3 = (pb, emit_l2(pb, ph0s))
                pend_l2 = (b, h0s)
            if pend_l3 is not None:
                emit_l3(*pend_l3)
            if pend_l2 is not None:
                pb, ph0s = pend_l2
                emit_l3(pb, emit_l2(pb, ph0s))

            # ---- entmax_bisect (last dim of size 1) over all tokens -------
            # weight = w32 + lin1_b;  alpha - 1 = sigmoid(weight) = 1/d
            t1 = chp.tile([NBLK, BLK], F32, tag="t1")
            nc.scalar.activation(t1[:], w32[:], AF.Exp,
                                 bias=lbbc[:], scale=-1.0)      # e^{-weight}
            dd = chp.tile([NBLK, BLK], F32, tag="dd")
            nc.vector.tensor_scalar_add(dd[:], t1[:], 1.0)      # 1/(alpha-1)
            rd = chp.tile([NBLK, BLK], F32, tag="rd")
            nc.vector.reciprocal(rd[:], dd[:])                  # alpha-1
            z = chp.tile([NBLK, BLK], F32, tag="z")
            nc.vector.scalar_tensor_tensor(z[:], s32[:], b3bc[:], rd[:],
                                           ALU.add, ALU.mult)   # s*(alpha-1)
            zm1 = chp.tile([NBLK, BLK], F32, tag="zm1")
            nc.vector.tensor_scalar_sub(zm1[:], z[:], 1.0)      # tau
            tq = chp.tile([NBLK, BLK], F32, tag="tq")
            nc.vector.tensor_tensor(tq[:], z[:], zm1[:], ALU.subtract)
            lq = chp.tile([NBLK, BLK], F32, tag="lq")
            nc.scalar.activation(lq[:], tq[:], AF.Ln)
            le = chp.tile([NBLK, BLK], F32, tag="le")
            nc.vector.tensor_tensor(le[:], lq[:], dd[:], ALU.mult)
            p = chp.tile([NBLK, BLK], F32, tag="p")
            nc.scalar.activation(p[:], le[:], AF.Exp)
            rp_ = chp.tile([NBLK, BLK], F32, tag="rp")
            nc.vector.reciprocal(rp_[:], p[:])
            res = chp.tile([NBLK, BLK], F32, tag="res")
            nc.vector.tensor_tensor(res[:], p[:], rp_[:], ALU.mult)

            nc.sync.dma_start(
                out_d[:].rearrange("(q t) o -> q (t o)", q=NBLK), res[:])
            if debug_sw:
                nc.sync.dma_start(dbg_d[0:NBLK, :], s32[:])
                nc.sync.dma_start(dbg_d[NBLK:2 * NBLK, :], w32[:])

    nc.compile()
    _CACHE[key] = nc
    return nc


def _prep_host(node1, u_rep, att1_w, att1_b, att2_w, att2_b, att3_w, att3_b,
               lin1_w, lin1_b):
    import ml_dtypes
    f32 = np.float32
    fp8 = ml_dtypes.float8_e4m3
    bf16 = ml_dtypes.bfloat16
    node1 = np.asarray(node1, f32)
    att1_w = np.asarray(att1_w, f32)
    att2_w = np.asarray(att2_w, f32)
    att3_w = np.asarray(att3_w, f32)
    lin1_w = np.asarray(lin1_w, f32)
    u_rep = np.asarray(u_rep, f32)
    C = np.float32(SC * A)

    # layer 1: u_rep's contribution + att1_b as per-feature bias u
    u_bias = (att1_w[:, D:] @ u_rep[0] + np.asarray(att1_b, f32)).astype(f32)
    w1at = np.ascontiguousarray(att1_w[:, :D].T).astype(fp8)   # [D, D]
    be1 = (u_bias + np.float32(LN_A)).reshape(D, 1)
    bnr1 = (-u_bias).reshape(D, 1)
    bpr1 = u_bias.reshape(D, 1).copy()

    # selu affine (selu = SC*nc - SC*A) folded into layer 2
    w2te = np.ascontiguousarray((SC * att2_w.T).astype(bf16))  # [D, 128]
    b2_eff = (np.asarray(att2_b, f32) - C * att2_w.sum(axis=1)).astype(f32)
    be2 = (b2_eff + np.float32(LN_A)).reshape(128, 1)
    bnr2 = (-b2_eff).reshape(128, 1)
    bpr2 = b2_eff.reshape(128, 1).copy()

    # selu affine folded into layer 3
    w3te = np.ascontiguousarray((SC * att3_w.T).astype(bf16))  # [128, 1]
    b3_eff = np.float32(np.asarray(att3_b, f32)[0] - C * att3_w.sum())

    lin1t = np.zeros((D, 16), f32)
    lin1t[:, 0] = lin1_w[0]
    lin1t = lin1t.astype(fp8)                                  # [D, 16] padded
    b3bc = np.full((NBLK, 1), b3_eff, f32)
    lbbc = np.full((NBLK, 1), -np.float32(np.asarray(lin1_b, f32)[0]), f32)

    shared = dict(w1at=w1at, lin1t=lin1t, w2te=w2te, w3te=w3te,
                  be1=np.ascontiguousarray(be1),
                  bnr1=np.ascontiguousarray(bnr1),
                  bpr1=np.ascontiguousarray(bpr1),
                  be2=np.ascontiguousarray(be2),
                  bnr2=np.ascontiguousarray(bnr2),
                  bpr2=np.ascontiguousarray(bpr2),
                  b3bc=b3bc, lbbc=lbbc)
    in_maps = []
    for c in range(N_CORES):
        m = dict(shared)
        m["n1t"] = np.ascontiguousarray(
            node1[c * TPC:(c + 1) * TPC, :].T).astype(fp8)
        in_maps.append(m)
    return in_maps


def kernel(node1, u_rep, att1_w, att1_b, att2_w, att2_b, att3_w, att3_b,
           lin1_w, lin1_b, num_neighs=None, **_unused):
    nc = _build()
    in_maps = _prep_host(node1, u_rep, att1_w, att1_b, att2_w, att2_b,
                         att3_w, att3_b, lin1_w, lin1_b)
    res = run_bass_kernel_spmd(nc, in_maps, core_ids=list(range(N_CORES)))
    out = np.concatenate([res.results[c]["out"] for c in range(N_CORES)],
                         axis=0)
    return out.astype(np.float32)


# revision 35
# speedup vs baseline: 7258.9460x; 1.1076x over previous
"""Trainium2 Bass kernel for nn_Attention_79998060855419 (sparse_attention).

Reference pipeline per row i of node1 [131072, 512]:
    x      = concat(node1[i], u_rep)                     # [1024]
    weight = node1[i] @ lin1_w.T + lin1_b                # [1]
    alpha  = sigmoid(weight) + 1                         # in (1, 2)
    h0     = selu(x @ att1_w.T + att1_b)                 # [512]
    h1     = selu(h0 @ att2_w.T + att2_b)                # [128]
    s      = h1 @ att3_w.T + att3_b                      # [1]
    out[i] = entmax_bisect(s, alpha)  over dim of size 1 # [1]

Distribution: pure data-parallel over the neighbor axis — 8 cores x 16384
rows; the tiny MLP weights and u_rep are replicated (per the sharding hint).
No collectives are needed; each core computes its shard's output.

Device-side dataflow (per core, 32 blocks of 512 tokens):
  - Activations flow transposed (features on partitions, tokens on the free
    axis): node1 is fed as node1.T tiles and every matmul is
    weights-stationary, so the M=1 reductions (lin1, att3) are cheap N=512
    matmuls and the entmax stage works on dense [32, 512] tiles.
  - Host prep only reshapes/transposes inputs and folds biases and the selu
    affine constants into downstream weights — all FLOPs over node1-derived
    data run on the NeuronCores.
  - Layer 1 (512x512) and lin1 run on the TensorEngine in fp8(e4m3) with
    perf_mode=DoubleRow (contraction packed in K-pairs, FD=512); layers 2/3
    run in bf16.  The final entmax normalization makes the output invariant
    to these precision choices (p/p == 1.0 bit-exactly either way).
  - selu(t), t = x + u (u = per-feature bias), is computed as
        e' = exp(x + (u + ln A))        (ScalarE, PSUM -> SBUF bf16)
        q  = min(e', A) + u             (VectorE tensor_scalar, bf16 4x)
        nc = max(x, -u) + q             (VectorE scalar_tensor_tensor)
    which equals selu(t)/SC + A; the affine map selu = SC*nc - SC*A is
    folded into the next layer's weights/bias on the host.
  - entmax_bisect with last-dim size 1 degenerates: tau_hi == tau_lo == z-1
    and dm0 == 0, so all 50 bisection iterations compute
    p = clip(z - (z-1), 0)^(1/(alpha-1)) and return p / sum(p) = p / p.
    The kernel computes exactly that (z = s*(alpha-1), t = z - (z-1),
    p = exp(ln(t) * 1/(alpha-1)), out = p * recip(p)) — numerically
    identical to running the fixed-point loop (the result is exactly 1.0
    for every finite positive p, on device and in the reference alike).
"""

import math

import numpy as np

import concourse.bacc as bacc
import concourse.mybir as mybir
import concourse.tile as tile
from concourse.bass_utils import run_bass_kernel_spmd

N = 131072
D = 512
N_CORES = 8
TPC = N // N_CORES          # tokens per core = 16384
BLK = 512                   # tokens per block
NBLK = TPC // BLK           # 32 blocks per core
NROW = NBLK

SC = 1.0507009873554804934193349852946   # selu scale
A = 1.6732632423543772848170429916717    # selu alpha
LN_A = math.log(A)

F32 = mybir.dt.float32
FP8 = mybir.dt.float8e4      # e4m3
BF16 = mybir.dt.bfloat16
AF = mybir.ActivationFunctionType
ALU = mybir.AluOpType
DR = mybir.MatmulPerfMode.DoubleRow

_CACHE = {}


def _build(nblk=NBLK, debug_sw=False):
    key = ("nc", nblk, debug_sw)
    if key in _CACHE:
        return _CACHE[key]

    nc = bacc.Bacc("TRN2", target_bir_lowering=False, debug=False,
                   num_devices=N_CORES)

    # Per-core inputs (shard of node1.T + replicated, host-folded weights).
    n1t_d = nc.dram_tensor("n1t", [D, TPC], FP8, kind="ExternalInput")
    w1at_d = nc.dram_tensor("w1at", [D, D], FP8, kind="ExternalInput")
    lin1t_d = nc.dram_tensor("lin1t", [D, 16], FP8, kind="ExternalInput")
    w2te_d = nc.dram_tensor("w2te", [D, 128], BF16, kind="ExternalInput")
    w3te_d = nc.dram_tensor("w3te", [128, 1], BF16, kind="ExternalInput")
    be1_d = nc.dram_tensor("be1", [D, 1], F32, kind="ExternalInput")
    bnr1_d = nc.dram_tensor("bnr1", [D, 1], F32, kind="ExternalInput")
    bpr1_d = nc.dram_tensor("bpr1", [D, 1], F32, kind="ExternalInput")
    be2_d = nc.dram_tensor("be2", [128, 1], F32, kind="ExternalInput")
    bnr2_d = nc.dram_tensor("bnr2", [128, 1], F32, kind="ExternalInput")
    bpr2_d = nc.dram_tensor("bpr2", [128, 1], F32, kind="ExternalInput")
    # per-partition broadcasts of scalar consts for the entmax chain
    b3bc_d = nc.dram_tensor("b3bc", [128, 1], F32, kind="ExternalInput")
    lbbc_d = nc.dram_tensor("lbbc", [128, 1], F32, kind="ExternalInput")
    ident_d = nc.dram_tensor("ident", [128, 128], F32, kind="ExternalInput")
    out_d = nc.dram_tensor("out", [TPC, 1], F32, kind="ExternalOutput")
    dbg_d = (nc.dram_tensor("dbg", [256, 4 * NBLK], F32, kind="ExternalOutput")
             if debug_sw else None)

    with tile.TileContext(nc) as tc:
        with (
            tc.tile_pool(name="wp", bufs=1) as wp,
            tc.tile_pool(name="n1p", bufs=3) as n1p,
            tc.tile_pool(name="ep", bufs=3) as ep,
            tc.tile_pool(name="rp", bufs=3) as rp,
            tc.tile_pool(name="h0p", bufs=8) as h0p,
            tc.tile_pool(name="h1p", bufs=2) as h1p,
            tc.tile_pool(name="swp", bufs=1) as swp,
            tc.tile_pool(name="stp", bufs=4) as stp,
            tc.tile_pool(name="chp", bufs=1) as chp,
            tc.tile_pool(name="ps1p", bufs=5, space="PSUM") as ps1p,
            tc.tile_pool(name="ps2p", bufs=1, space="PSUM") as ps2p,
            tc.tile_pool(name="pssp", bufs=1, space="PSUM") as pssp,
            tc.tile_pool(name="pswp", bufs=1, space="PSUM") as pswp,
        ):
            # ---- replicated weights / biases in SBUF ----------------------
            w1a = wp.tile([128, 4, D], FP8, tag="w1a")
            nc.sync.dma_start(
                w1a[:], w1at_d[:].rearrange("(k p) m -> p k m", p=128))
            lin1 = wp.tile([128, 4, 16], FP8, tag="lin1")
            nc.sync.dma_start(
                lin1[:], lin1t_d[:].rearrange("(k p) o -> p k o", p=128))
            w2 = wp.tile([128, 4 * 128], BF16, tag="w2")
            nc.sync.dma_start(
                w2[:], w2te_d[:].rearrange("(k p) m -> p k m", p=128))
            w3 = wp.tile([128, 1], BF16, tag="w3")
            nc.sync.dma_start(w3[:], w3te_d[:])
            be1 = wp.tile([128, 4], F32, tag="be1")
            nc.sync.dma_start(
                be1[:], be1_d[:].rearrange("(k p) o -> p k o", p=128))
            bnr1 = wp.tile([128, 4], F32, tag="bnr1")
            nc.sync.dma_start(
                bnr1[:], bnr1_d[:].rearrange("(k p) o -> p k o", p=128))
            bpr1 = wp.tile([128, 4], F32, tag="bpr1")
            nc.sync.dma_start(
                bpr1[:], bpr1_d[:].rearrange("(k p) o -> p k o", p=128))
            be2 = wp.tile([128, 1], F32, tag="be2")
            nc.sync.dma_start(be2[:], be2_d[:])
            bnr2 = wp.tile([128, 1], F32, tag="bnr2")
            nc.sync.dma_start(bnr2[:], bnr2_d[:])
            bpr2 = wp.tile([128, 1], F32, tag="bpr2")
            nc.sync.dma_start(bpr2[:], bpr2_d[:])
            b3bc = wp.tile([128, 1], F32, tag="b3bc")
            nc.sync.dma_start(b3bc[:], b3bc_d[:])
            lbbc = wp.tile([128, 1], F32, tag="lbbc")
            nc.sync.dma_start(lbbc[:], lbbc_d[:])
            ident = wp.tile([128, 128], F32, tag="ident")
            nc.sync.dma_start(ident[:], ident_d[:])

            # s / w accumulate directly in PSUM via tokens-as-M (M=128, N=1)
            # matmuls: column 4*b+j holds tokens [b*512+j*128, ...+128).
            sAcc = pssp.tile([128, 4 * NBLK], F32, tag="sAcc")
            wAcc = pswp.tile([128, 4 * NBLK], F32, tag="wAcc")

            # ---- per-block emitters (software-pipelined below) ------------
            def emit_l1(b):
                n1 = n1p.tile([128, 4, BLK], FP8, tag="n1")
                nc.sync.dma_start(
                    n1[:],
                    n1t_d[:, b * BLK:(b + 1) * BLK]
                    .rearrange("(k p) t -> p k t", p=128))
                h0s = []
                for m in range(4):
                    ps1 = ps1p.tile([128, BLK], F32, tag="ps1")
                    for j in range(2):       # DoubleRow K pairs (K=2x128)
                        nc.tensor.matmul(
                            ps1[:],
                            w1a[:, 2 * j:2 * j + 2, m * 128:(m + 1) * 128],
                            n1[:, 2 * j:2 * j + 2, :],
                            perf_mode=DR, start=(j == 0), stop=(j == 1))
                    e = ep.tile([128, BLK], BF16, tag="e")
                    nc.scalar.activation(e[:], ps1[:], AF.Exp,
                                         bias=be1[:, m:m + 1])
                    q = rp.tile([128, BLK], BF16, tag="q")
                    nc.vector.tensor_scalar(q[:], e[:],
                                            A, bpr1[:, m:m + 1],
                                            ALU.min, ALU.add)
                    h0 = h0p.tile([128, BLK], BF16, tag="h0")
                    nc.vector.scalar_tensor_tensor(h0[:], ps1[:],
                                                   bnr1[:, m:m + 1], q[:],
                                                   ALU.max, ALU.add)
                    h0s.append(h0)
                for t in range(4):       # token subtiles as M
                    col = 4 * b + t
                    for j in range(2):
                        nc.tensor.matmul(
                            wAcc[:, col:col + 1],
                            n1[:, 2 * j:2 * j + 2, t * 128:(t + 1) * 128],
                            lin1[:, 2 * j:2 * j + 2, 0:1],
                            perf_mode=DR, start=(j == 0), stop=(j == 1))
                return h0s

            def emit_l2(b, h0s):
                ps2 = ps2p.tile([128, BLK], F32, tag="ps2")
                for k in range(4):
                    nc.tensor.matmul(ps2[:], w2[:, k * 128:(k + 1) * 128],
                                     h0s[k][:], start=(k == 0), stop=(k == 3))
                e2 = ep.tile([128, BLK], BF16, tag="e2")
                nc.scalar.activation(e2[:], ps2[:], AF.Exp, bias=be2[:])
                r2 = rp.tile([128, BLK], BF16, tag="r2")
                nc.scalar.activation(r2[:], ps2[:], AF.Relu, bias=bpr2[:])
                q2 = rp.tile([128, BLK], BF16, tag="q2")
                nc.vector.tensor_scalar_min(q2[:], e2[:], A)
                h1 = h1p.tile([128, BLK], BF16, tag="h1")
                nc.vector.tensor_tensor(h1[:], r2[:], q2[:], ALU.add)
                return h1

            def emit_l3(b, h1):
                for t in range(4):       # token subtiles as M
                    col = 4 * b + t
                    nc.tensor.matmul(sAcc[:, col:col + 1],
                                     h1[:, t * 128:(t + 1) * 128], w3[:],
                                     start=True, stop=True)

            # PE executes its queue in order: L2 of block b-1 and L3 of block
            # b-2 are emitted under L1 of block b, so the PE never waits on
            # the ACT/DVE selu chains.
            pend_l2 = None
            pend_l3 = None
            for b in range(nblk):
                h0s = emit_l1(b)
                if pend_l3 is not None:
                    emit_l3(*pend_l3)
                    pend_l3 = None
                if pend_l2 is not None:
                    pb, ph0s = pend_l2
                    pend_l3 = (pb, emit_l2(pb, ph0s))
                pend_l2 = (b, h0s)
            if pend_l3 is not None:
                emit_l3(*pend_l3)
            if pend_l2 is not None:
                pb, ph0s = pend_l2
                emit_l3(pb, emit_l2(pb, ph0s))

            # ---- entmax_bisect (last dim of size 1) over all tokens -------
            # weight = wAcc + lin1_b;  alpha - 1 = sigmoid(weight) = 1/d
            CC = 4 * NBLK
            t1 = chp.tile([128, CC], F32, tag="t1")
            nc.scalar.activation(t1[:], wAcc[:], AF.Exp,
                                 bias=lbbc[:], scale=-1.0)      # e^{-weight}
            dd = chp.tile([128, CC], F32, tag="dd")
            nc.vector.tensor_scalar_add(dd[:], t1[:], 1.0)      # 1/(alpha-1)
            rd = chp.tile([128, CC], F32, tag="rd")
            nc.vector.reciprocal(rd[:], dd[:])                  # alpha-1
            z = chp.tile([128, CC], F32, tag="z")
            nc.vector.scalar_tensor_tensor(z[:], sAcc[:], b3bc[:], rd[:],
                                           ALU.add, ALU.mult)   # s*(alpha-1)
            zm1 = chp.tile([128, CC], F32, tag="zm1")
            nc.vector.tensor_scalar_sub(zm1[:], z[:], 1.0)      # tau
            tq = chp.tile([128, CC], F32, tag="tq")
            nc.vector.tensor_tensor(tq[:], z[:], zm1[:], ALU.subtract)
            lq = chp.tile([128, CC], F32, tag="lq")
            nc.scalar.activation(lq[:], tq[:], AF.Ln)
            le = chp.tile([128, CC], F32, tag="le")
            nc.vector.tensor_tensor(le[:], lq[:], dd[:], ALU.mult)
            p = chp.tile([128, CC], F32, tag="p")
            nc.scalar.activation(p[:], le[:], AF.Exp)
            rp_ = chp.tile([128, CC], F32, tag="rp")
            nc.vector.reciprocal(rp_[:], p[:])
            res = chp.tile([128, CC], F32, tag="res")
            nc.vector.tensor_tensor(res[:], p[:], rp_[:], ALU.mult)

            # res[p, c] = token c*128 + p -> transpose so partition c holds
            # 128 contiguous tokens, then one dense store.
            rest = ps1p.tile([128, 128], F32, tag="ps1")
            nc.tensor.transpose(rest[:], res[:], ident[:])
            resT = chp.tile([128, 128], F32, tag="resT")
            nc.scalar.copy(resT[:], rest[:])
            nc.sync.dma_start(
                out_d[:].rearrange("(c p) o -> c (p o)", c=128), resT[:])
            if debug_sw:
                sdbg = chp.tile([128, CC], F32, tag="sdbg")
                nc.scalar.copy(sdbg[:], sAcc[:])
                wdbg = chp.tile([128, CC], F32, tag="wdbg")
                nc.scalar.copy(wdbg[:], wAcc[:])
                nc.sync.dma_start(dbg_d[0:128, :], sdbg[:])
                nc.sync.dma_start(dbg_d[128:256, :], wdbg[:])

    nc.compile()
    _CACHE[key] = nc
    return nc


def _prep_host(node1, u_rep, att1_w, att1_b, att2_w, att2_b, att3_w, att3_b,
               lin1_w, lin1_b):
    import ml_dtypes
    f32 = np.float32
    fp8 = ml_dtypes.float8_e4m3
    bf16 = ml_dtypes.bfloat16
    node1 = np.asarray(node1, f32)
    att1_w = np.asarray(att1_w, f32)
    att2_w = np.asarray(att2_w, f32)
    att3_w = np.asarray(att3_w, f32)
    lin1_w = np.asarray(lin1_w, f32)
    u_rep = np.asarray(u_rep, f32)
    C = np.float32(SC * A)

    # layer 1: u_rep's contribution + att1_b as per-feature bias u
    u_bias = (att1_w[:, D:] @ u_rep[0] + np.asarray(att1_b, f32)).astype(f32)
    w1at = np.ascontiguousarray(att1_w[:, :D].T).astype(fp8)   # [D, D]
    be1 = (u_bias + np.float32(LN_A)).reshape(D, 1)
    bnr1 = (-u_bias).reshape(D, 1)
    bpr1 = u_bias.reshape(D, 1).copy()

    # selu affine (selu = SC*nc - SC*A) folded into layer 2
    w2te = np.ascontiguousarray((SC * att2_w.T).astype(bf16))  # [D, 128]
    b2_eff = (np.asarray(att2_b, f32) - C * att2_w.sum(axis=1)).astype(f32)
    be2 = (b2_eff + np.float32(LN_A)).reshape(128, 1)
    bnr2 = (-b2_eff).reshape(128, 1)
    bpr2 = b2_eff.reshape(128, 1).copy()

    # selu affine folded into layer 3
    w3te = np.ascontiguousarray((SC * att3_w.T).astype(bf16))  # [128, 1]
    b3_eff = np.float32(np.asarray(att3_b, f32)[0] - C * att3_w.sum())

    lin1t = np.zeros((D, 16), f32)
    lin1t[:, 0] = lin1_w[0]
    lin1t = lin1t.astype(fp8)                                  # [D, 16] padded
    b3bc = np.full((128, 1), b3_eff, f32)
    lbbc = np.full((128, 1), -np.float32(np.asarray(lin1_b, f32)[0]), f32)
    ident = np.eye(128, dtype=f32)

    shared = dict(w1at=w1at, lin1t=lin1t, w2te=w2te, w3te=w3te,
                  be1=np.ascontiguousarray(be1),
                  bnr1=np.ascontiguousarray(bnr1),
                  bpr1=np.ascontiguousarray(bpr1),
                  be2=np.ascontiguousarray(be2),
                  bnr2=np.ascontiguousarray(bnr2),
                  bpr2=np.ascontiguousarray(bpr2),
                  b3bc=b3bc, lbbc=lbbc, ident=ident)
    in_maps = []
    for c in range(N_CORES):
        m = dict(shared)
        m["n1t"] = np.ascontiguousarray(
            node1[c * TPC:(c + 1) * TPC, :].T).astype(fp8)
        in_maps.append(m)
    return in_maps


def kernel(node1, u_rep, att1_w, att1_b, att2_w, att2_b, att3_w, att3_b,
           lin1_w, lin1_b, num_neighs=None, **_unused):
    nc = _build()
    in_maps = _prep_host(node1, u_rep, att1_w, att1_b, att2_w, att2_b,
                         att3_w, att3_b, lin1_w, lin1_b)
    res = run_bass_kernel_spmd(nc, in_maps, core_ids=list(range(N_CORES)))
    out = np.concatenate([res.results[c]["out"] for c in range(N_CORES)],
                         axis=0)
    return out.astype(np.float32)


# revision 36
# speedup vs baseline: 7655.3410x; 1.0546x over previous
"""Trainium2 Bass kernel for nn_Attention_79998060855419 (sparse_attention).

Reference pipeline per row i of node1 [131072, 512]:
    x      = concat(node1[i], u_rep)                     # [1024]
    weight = node1[i] @ lin1_w.T + lin1_b                # [1]
    alpha  = sigmoid(weight) + 1                         # in (1, 2)
    h0     = selu(x @ att1_w.T + att1_b)                 # [512]
    h1     = selu(h0 @ att2_w.T + att2_b)                # [128]
    s      = h1 @ att3_w.T + att3_b                      # [1]
    out[i] = entmax_bisect(s, alpha)  over dim of size 1 # [1]

Distribution: pure data-parallel over the neighbor axis — 8 cores x 16384
rows; the tiny MLP weights and u_rep are replicated (per the sharding hint).
No collectives are needed; each core computes its shard's output.

Device-side dataflow (per core, 32 blocks of 512 tokens):
  - Activations flow transposed (features on partitions, tokens on the free
    axis): node1 is fed as node1.T tiles and every matmul is
    weights-stationary, so the M=1 reductions (lin1, att3) are cheap N=512
    matmuls and the entmax stage works on dense [32, 512] tiles.
  - Host prep only reshapes/transposes inputs and folds biases and the selu
    affine constants into downstream weights — all FLOPs over node1-derived
    data run on the NeuronCores.
  - Layer 1 (512x512) and lin1 run on the TensorEngine in fp8(e4m3) with
    perf_mode=DoubleRow (contraction packed in K-pairs, FD=512); layers 2/3
    run in bf16.  The final entmax normalization makes the output invariant
    to these precision choices (p/p == 1.0 bit-exactly either way).
  - selu(t), t = x + u (u = per-feature bias), is computed as
        e' = exp(x + (u + ln A))        (ScalarE, PSUM -> SBUF bf16)
        q  = min(e', A) + u             (VectorE tensor_scalar, bf16 4x)
        nc = max(x, -u) + q             (VectorE scalar_tensor_tensor)
    which equals selu(t)/SC + A; the affine map selu = SC*nc - SC*A is
    folded into the next layer's weights/bias on the host.
  - entmax_bisect with last-dim size 1 degenerates: tau_hi == tau_lo == z-1
    and dm0 == 0, so all 50 bisection iterations compute
    p = clip(z - (z-1), 0)^(1/(alpha-1)) and return p / sum(p) = p / p.
    The kernel computes exactly that (z = s*(alpha-1), t = z - (z-1),
    p = exp(ln(t) * 1/(alpha-1)), out = p * recip(p)) — numerically
    identical to running the fixed-point loop (the result is exactly 1.0
    for every finite positive p, on device and in the reference alike).
"""

import math

import numpy as np

import concourse.bacc as bacc
import concourse.mybir as mybir
import concourse.tile as tile
from concourse.bass_utils import run_bass_kernel_spmd

N = 131072
D = 512
N_CORES = 8
TPC = N // N_CORES          # tokens per core = 16384
BLK = 512                   # tokens per block
NBLK = TPC // BLK           # 32 blocks per core
NROW = NBLK

SC = 1.0507009873554804934193349852946   # selu scale
A = 1.6732632423543772848170429916717    # selu alpha
LN_A = math.log(A)

F32 = mybir.dt.float32
FP8 = mybir.dt.float8e4      # e4m3
BF16 = mybir.dt.bfloat16
AF = mybir.ActivationFunctionType
ALU = mybir.AluOpType
DR = mybir.MatmulPerfMode.DoubleRow

_CACHE = {}


def _build(nblk=NBLK, debug_sw=False):
    key = ("nc", nblk, debug_sw)
    if key in _CACHE:
        return _CACHE[key]

    nc = bacc.Bacc("TRN2", target_bir_lowering=False, debug=False,
                   num_devices=N_CORES)

    # Per-core inputs (shard of node1.T + replicated, host-folded weights).
    n1t_d = nc.dram_tensor("n1t", [D, TPC], FP8, kind="ExternalInput")
    w1at_d = nc.dram_tensor("w1at", [D, D], FP8, kind="ExternalInput")
    lin1t_d = nc.dram_tensor("lin1t", [D, 16], FP8, kind="ExternalInput")
    w2te_d = nc.dram_tensor("w2te", [D, 128], BF16, kind="ExternalInput")
    w3te_d = nc.dram_tensor("w3te", [128, 1], BF16, kind="ExternalInput")
    ub_d = nc.dram_tensor("ub", [1, D], BF16, kind="ExternalInput")
    be2_d = nc.dram_tensor("be2", [128, 1], F32, kind="ExternalInput")
    bpr2_d = nc.dram_tensor("bpr2", [128, 1], F32, kind="ExternalInput")
    # per-partition broadcasts of scalar consts for the entmax chain
    b3bc_d = nc.dram_tensor("b3bc", [128, 1], F32, kind="ExternalInput")
    lbbc_d = nc.dram_tensor("lbbc", [128, 1], F32, kind="ExternalInput")
    ident_d = nc.dram_tensor("ident", [128, 128], F32, kind="ExternalInput")
    out_d = nc.dram_tensor("out", [TPC, 1], F32, kind="ExternalOutput")
    dbg_d = (nc.dram_tensor("dbg", [256, 4 * NBLK], F32, kind="ExternalOutput")
             if debug_sw else None)

    with tile.TileContext(nc) as tc:
        with (
            tc.tile_pool(name="wp", bufs=1) as wp,
            tc.tile_pool(name="n1p", bufs=3) as n1p,
            tc.tile_pool(name="ep", bufs=3) as ep,
            tc.tile_pool(name="rp", bufs=3) as rp,
            tc.tile_pool(name="h0p", bufs=8) as h0p,
            tc.tile_pool(name="h1p", bufs=2) as h1p,
            tc.tile_pool(name="swp", bufs=1) as swp,
            tc.tile_pool(name="stp", bufs=4) as stp,
            tc.tile_pool(name="chp", bufs=1) as chp,
            tc.tile_pool(name="ps1p", bufs=3, space="PSUM") as ps1p,
            tc.tile_pool(name="ps2p", bufs=1, space="PSUM") as ps2p,
            tc.tile_pool(name="pssp", bufs=1, space="PSUM") as pssp,
        ):
            # ---- replicated weights / biases in SBUF ----------------------
            w1a = wp.tile([128, 4, D], FP8, tag="w1a")
            nc.sync.dma_start(
                w1a[:], w1at_d[:].rearrange("(k p) m -> p k m", p=128))
            lin1 = wp.tile([128, 4, 16], FP8, tag="lin1")
            nc.sync.dma_start(
                lin1[:], lin1t_d[:].rearrange("(k p) o -> p k o", p=128))
            w2 = wp.tile([128, 4 * 128], BF16, tag="w2")
            nc.sync.dma_start(
                w2[:], w2te_d[:].rearrange("(k p) m -> p k m", p=128))
            w3 = wp.tile([128, 1], BF16, tag="w3")
            nc.sync.dma_start(w3[:], w3te_d[:])
            ub = wp.tile([1, D], BF16, tag="ub")
            nc.sync.dma_start(ub[:], ub_d[:])
            ones = wp.tile([1, BLK], BF16, tag="ones")
            nc.vector.memset(ones[:], 1.0)
            lna = wp.tile([128, 1], F32, tag="lna")
            nc.vector.memset(lna[:], LN_A)
            be2 = wp.tile([128, 1], F32, tag="be2")
            nc.sync.dma_start(be2[:], be2_d[:])
            bpr2 = wp.tile([128, 1], F32, tag="bpr2")
            nc.sync.dma_start(bpr2[:], bpr2_d[:])
            b3bc = wp.tile([128, 1], F32, tag="b3bc")
            nc.sync.dma_start(b3bc[:], b3bc_d[:])
            lbbc = wp.tile([128, 1], F32, tag="lbbc")
            nc.sync.dma_start(lbbc[:], lbbc_d[:])
            ident = wp.tile([128, 128], F32, tag="ident")
            nc.sync.dma_start(ident[:], ident_d[:])

            # s / w accumulate directly in PSUM via tokens-as-M (M=128, N=1)
            # matmuls: column 4*b+j holds tokens [b*512+j*128, ...+128).
            swAcc = pssp.tile([128, 8 * NBLK], F32, tag="swAcc")
            sAcc = swAcc[:, 0:4 * NBLK]
            wAcc = swAcc[:, 4 * NBLK:8 * NBLK]

            # ---- per-block emitters (software-pipelined below) ------------
            def emit_l1(b):
                n1 = n1p.tile([128, 4, BLK], FP8, tag="n1")
                nc.sync.dma_start(
                    n1[:],
                    n1t_d[:, b * BLK:(b + 1) * BLK]
                    .rearrange("(k p) t -> p k t", p=128))
                h0s = []
                for pair in range(2):    # m-chunk pairs: (0,1) and (2,3)
                    ps1 = ps1p.tile([128, 2, BLK], F32, tag="ps1")
                    for mi in range(2):
                        m = 2 * pair + mi
                        for j in range(2):   # DoubleRow K pairs (K=2x128)
                            nc.tensor.matmul(
                                ps1[:, mi, :],
                                w1a[:, 2 * j:2 * j + 2,
                                    m * 128:(m + 1) * 128],
                                n1[:, 2 * j:2 * j + 2, :],
                                perf_mode=DR, start=(j == 0), stop=False)
                        # add the per-feature bias u exactly (bf16 rank-1)
                        nc.tensor.matmul(
                            ps1[:, mi, :],
                            ub[:, m * 128:(m + 1) * 128], ones[:],
                            start=False, stop=True)
                    e = ep.tile([128, 2 * BLK], BF16, tag="e")
                    nc.scalar.activation(e[:], ps1[:], AF.Exp, bias=lna[:])
                    q = rp.tile([128, 2 * BLK], BF16, tag="q")
                    nc.vector.tensor_scalar_min(q[:], e[:], A)
                    h0 = h0p.tile([128, 2 * BLK], BF16, tag="h0")
                    nc.vector.scalar_tensor_tensor(h0[:], ps1[:], 0.0, q[:],
                                                   ALU.max, ALU.add)
                    h0s.append(h0)
                for t in range(4):       # token subtiles as M
                    col = 4 * b + t
                    for j in range(2):
                        nc.tensor.matmul(
                            wAcc[:, col:col + 1],
                            n1[:, 2 * j:2 * j + 2, t * 128:(t + 1) * 128],
                            lin1[:, 2 * j:2 * j + 2, 0:1],
                            perf_mode=DR, start=(j == 0), stop=(j == 1))
                return h0s

            def emit_l2(b, h0s):
                ps2 = ps2p.tile([128, BLK], F32, tag="ps2")
                for k in range(4):
                    nc.tensor.matmul(
                        ps2[:], w2[:, k * 128:(k + 1) * 128],
                        h0s[k // 2][:, (k % 2) * BLK:(k % 2 + 1) * BLK],
                        start=(k == 0), stop=(k == 3))
                e2 = ep.tile([128, BLK], BF16, tag="e2")
                nc.scalar.activation(e2[:], ps2[:], AF.Exp, bias=be2[:])
                r2 = rp.tile([128, BLK], BF16, tag="r2")
                nc.scalar.activation(r2[:], ps2[:], AF.Relu, bias=bpr2[:])
                q2 = rp.tile([128, BLK], BF16, tag="q2")
                nc.vector.tensor_scalar_min(q2[:], e2[:], A)
                h1 = h1p.tile([128, BLK], BF16, tag="h1")
                nc.vector.tensor_tensor(h1[:], r2[:], q2[:], ALU.add)
                return h1

            def emit_l3(b, h1):
                for t in range(4):       # token subtiles as M
                    col = 4 * b + t
                    nc.tensor.matmul(sAcc[:, col:col + 1],
                                     h1[:, t * 128:(t + 1) * 128], w3[:],
                                     start=True, stop=True)

            # PE executes its queue in order: L2 of block b-1 and L3 of block
            # b-2 are emitted under L1 of block b, so the PE never waits on
            # the ACT/DVE selu chains.
            pend_l2 = None
            pend_l3 = None
            for b in range(nblk):
                h0s = emit_l1(b)
                if pend_l3 is not None:
                    emit_l3(*pend_l3)
                    pend_l3 = None
                if pend_l2 is not None:
                    pb, ph0s = pend_l2
                    pend_l3 = (pb, emit_l2(pb, ph0s))
                pend_l2 = (b, h0s)
            if pend_l3 is not None:
                emit_l3(*pend_l3)
            if pend_l2 is not None:
                pb, ph0s = pend_l2
                emit_l3(pb, emit_l2(pb, ph0s))

            # ---- entmax_bisect (last dim of size 1) over all tokens -------
            # weight = wAcc + lin1_b;  alpha - 1 = sigmoid(weight) = 1/d
            CC = 4 * NBLK
            t1 = chp.tile([128, CC], F32, tag="t1")
            nc.scalar.activation(t1[:], wAcc[:], AF.Exp,
                                 bias=lbbc[:], scale=-1.0)      # e^{-weight}
            dd = chp.tile([128, CC], F32, tag="dd")
            nc.vector.tensor_scalar_add(dd[:], t1[:], 1.0)      # 1/(alpha-1)
            rd = chp.tile([128, CC], F32, tag="rd")
            nc.vector.reciprocal(rd[:], dd[:])                  # alpha-1
            z = chp.tile([128, CC], F32, tag="z")
            nc.vector.scalar_tensor_tensor(z[:], sAcc[:], b3bc[:], rd[:],
                                           ALU.add, ALU.mult)   # s*(alpha-1)
            zm1 = chp.tile([128, CC], F32, tag="zm1")
            nc.vector.tensor_scalar_sub(zm1[:], z[:], 1.0)      # tau
            tq = chp.tile([128, CC], F32, tag="tq")
            nc.vector.tensor_tensor(tq[:], z[:], zm1[:], ALU.subtract)
            lq = chp.tile([128, CC], F32, tag="lq")
            nc.scalar.activation(lq[:], tq[:], AF.Ln)
            le = chp.tile([128, CC], F32, tag="le")
            nc.vector.tensor_tensor(le[:], lq[:], dd[:], ALU.mult)
            p = chp.tile([128, CC], F32, tag="p")
            nc.scalar.activation(p[:], le[:], AF.Exp)
            rp_ = chp.tile([128, CC], F32, tag="rp")
            nc.vector.reciprocal(rp_[:], p[:])
            res = chp.tile([128, CC], F32, tag="res")
            nc.vector.tensor_tensor(res[:], p[:], rp_[:], ALU.mult)

            # res[p, c] = token c*128 + p -> transpose so partition c holds
            # 128 contiguous tokens, then one dense store.
            rest = ps1p.tile([128, 128], F32, tag="ps1")
            nc.tensor.transpose(rest[:], res[:], ident[:])
            resT = chp.tile([128, 128], F32, tag="resT")
            nc.scalar.copy(resT[:], rest[:])
            nc.sync.dma_start(
                out_d[:].rearrange("(c p) o -> c (p o)", c=128), resT[:])
            if debug_sw:
                sdbg = chp.tile([128, CC], F32, tag="sdbg")
                nc.scalar.copy(sdbg[:], sAcc[:])
                wdbg = chp.tile([128, CC], F32, tag="wdbg")
                nc.scalar.copy(wdbg[:], wAcc[:])
                nc.sync.dma_start(dbg_d[0:128, :], sdbg[:])
                nc.sync.dma_start(dbg_d[128:256, :], wdbg[:])

    nc.compile()
    _CACHE[key] = nc
    return nc


def _prep_host(node1, u_rep, att1_w, att1_b, att2_w, att2_b, att3_w, att3_b,
               lin1_w, lin1_b):
    import ml_dtypes
    f32 = np.float32
    fp8 = ml_dtypes.float8_e4m3
    bf16 = ml_dtypes.bfloat16
    node1 = np.asarray(node1, f32)
    att1_w = np.asarray(att1_w, f32)
    att2_w = np.asarray(att2_w, f32)
    att3_w = np.asarray(att3_w, f32)
    lin1_w = np.asarray(lin1_w, f32)
    u_rep = np.asarray(u_rep, f32)
    C = np.float32(SC * A)

    # layer 1: u_rep's contribution + att1_b as per-feature bias u
    u_bias = (att1_w[:, D:] @ u_rep[0] + np.asarray(att1_b, f32)).astype(f32)
    w1at = np.ascontiguousarray(att1_w[:, :D].T).astype(fp8)   # [D, D]
    ub = np.ascontiguousarray(u_bias.reshape(1, D)).astype(bf16)

    # selu affine (selu = SC*nc - SC*A) folded into layer 2
    w2te = np.ascontiguousarray((SC * att2_w.T).astype(bf16))  # [D, 128]
    b2_eff = (np.asarray(att2_b, f32) - C * att2_w.sum(axis=1)).astype(f32)
    be2 = (b2_eff + np.float32(LN_A)).reshape(128, 1)
    bpr2 = b2_eff.reshape(128, 1).copy()

    # selu affine folded into layer 3
    w3te = np.ascontiguousarray((SC * att3_w.T).astype(bf16))  # [128, 1]
    b3_eff = np.float32(np.asarray(att3_b, f32)[0] - C * att3_w.sum())

    lin1t = np.zeros((D, 16), f32)
    lin1t[:, 0] = lin1_w[0]
    lin1t = lin1t.astype(fp8)                                  # [D, 16] padded
    b3bc = np.full((128, 1), b3_eff, f32)
    lbbc = np.full((128, 1), -np.float32(np.asarray(lin1_b, f32)[0]), f32)
    ident = np.eye(128, dtype=f32)

    shared = dict(w1at=w1at, lin1t=lin1t, ub=ub, w2te=w2te, w3te=w3te,
                  be2=np.ascontiguousarray(be2),
                  bpr2=np.ascontiguousarray(bpr2),
                  b3bc=b3bc, lbbc=lbbc, ident=ident)
    in_maps = []
    for c in range(N_CORES):
        m = dict(shared)
        m["n1t"] = np.ascontiguousarray(
            node1[c * TPC:(c + 1) * TPC, :].T).astype(fp8)
        in_maps.append(m)
    return in_maps


def kernel(node1, u_rep, att1_w, att1_b, att2_w, att2_b, att3_w, att3_b,
           lin1_w, lin1_b, num_neighs=None, **_unused):
    nc = _build()
    in_maps = _prep_host(node1, u_rep, att1_w, att1_b, att2_w, att2_b,
                         att3_w, att3_b, lin1_w, lin1_b)
    res = run_bass_kernel_spmd(nc, in_maps, core_ids=list(range(N_CORES)))
    out = np.concatenate([res.results[c]["out"] for c in range(N_CORES)],
                         axis=0)
    return out.astype(np.float32)


# revision 52
# speedup vs baseline: 8256.4583x; 1.0785x over previous
"""Trainium2 Bass kernel for nn_Attention_79998060855419 (sparse_attention).

Reference pipeline per row i of node1 [131072, 512]:
    x      = concat(node1[i], u_rep)                     # [1024]
    weight = node1[i] @ lin1_w.T + lin1_b                # [1]
    alpha  = sigmoid(weight) + 1                         # in (1, 2)
    h0     = selu(x @ att1_w.T + att1_b)                 # [512]
    h1     = selu(h0 @ att2_w.T + att2_b)                # [128]
    s      = h1 @ att3_w.T + att3_b                      # [1]
    out[i] = entmax_bisect(s, alpha)  over dim of size 1 # [1]

Distribution: pure data-parallel over the neighbor axis — 8 cores x 16384
rows; the tiny MLP weights and u_rep are replicated (per the sharding hint).
No collectives are needed; each core computes its shard's output.

Device-side dataflow (per core, 32 blocks of 512 tokens):
  - Activations flow transposed (features on partitions, tokens on the free
    axis): node1 is fed as node1.T tiles and the layer matmuls are
    weights-stationary.  The row-reductions (lin1, att3) run tokens-as-M
    (M=128, N=1) and accumulate straight into a persistent PSUM tile, so
    the entmax stage is one [128, 128] pass with no staging copies.
  - Host prep only reshapes/transposes inputs and folds biases and the selu
    affine constants into downstream weights — all FLOPs over node1-derived
    data run on the NeuronCores.
  - Layer 1 (512x512) and lin1 run on the TensorEngine in fp8(e4m3) with
    perf_mode=DoubleRow (contraction packed in K-pairs, FD=512); layers 2/3
    run in bf16.  The final entmax normalization makes the output invariant
    to these precision choices (p/p == 1.0 bit-exactly either way).
  - selu(t): the per-feature bias u enters the PSUM through an exact bf16
    rank-1 (u x ones) K=1 matmul in the same accumulation group, so t sits
    in PSUM and every selu scalar is a constant; layer-1 m-chunks are
    processed as [128, 2, 512] two-bank PSUM pairs:
        e' = exp(t + ln A)              (ScalarE, PSUM -> SBUF bf16)
        q  = min(e', A)                 (VectorE tensor_scalar, bf16 4x)
        nc = max(t, 0) + q              (VectorE scalar_tensor_tensor)
    which equals selu(t)/SC + A; the affine map selu = SC*nc - SC*A is
    folded into the next layer's weights/bias on the host.
  - entmax_bisect with last-dim size 1 degenerates: tau_hi == tau_lo == z-1
    and dm0 == 0, so all 50 bisection iterations compute
    p = clip(z - (z-1), 0)^(1/(alpha-1)) and return p / sum(p) = p / p.
    The kernel computes exactly that: z = s*(alpha-1), t = z - (z-1)
    (so |t-1| <= ulp(1)), p = t^(1/(alpha-1)) evaluated via the
    first-order forms ln(t) = t-1 and exp(x) = 1+x — bit-exact in f32 for
    this value range since the quadratic terms sit below half-ulp — and
    out = p * recip(p).  The result is exactly 1.0 for every finite
    positive p, on device and in the reference alike.
"""

import math

import numpy as np

import concourse.bacc as bacc
import concourse.mybir as mybir
import concourse.tile as tile
from concourse.bass_utils import run_bass_kernel_spmd

N = 131072
D = 512
N_CORES = 8
TPC = N // N_CORES          # tokens per core = 16384
BLK = 512                   # tokens per block
NBLK = TPC // BLK           # 32 blocks per core
NROW = NBLK

SC = 1.0507009873554804934193349852946   # selu scale
A = 1.6732632423543772848170429916717    # selu alpha
LN_A = math.log(A)

F32 = mybir.dt.float32
FP8 = mybir.dt.float8e4      # e4m3
BF16 = mybir.dt.bfloat16
AF = mybir.ActivationFunctionType
ALU = mybir.AluOpType
DR = mybir.MatmulPerfMode.DoubleRow

_CACHE = {}


def _build(nblk=NBLK, debug_sw=False):
    key = ("nc", nblk, debug_sw)
    if key in _CACHE:
        return _CACHE[key]

    nc = bacc.Bacc("TRN2", target_bir_lowering=False, debug=False,
                   num_devices=N_CORES)

    # Per-core inputs (shard of node1.T + replicated, host-folded weights).
    # block-major node1.T: [block, partition, k-chunk * tokens], so each
    # block load is one fully contiguous 2KB-per-partition DMA
    n1t_d = nc.dram_tensor("n1t", [NBLK, 128, 4 * BLK], FP8,
                           kind="ExternalInput")
    w1at_d = nc.dram_tensor("w1at", [D, D], FP8, kind="ExternalInput")
    lin1t_d = nc.dram_tensor("lin1t", [D, 16], FP8, kind="ExternalInput")
    w2te_d = nc.dram_tensor("w2te", [D, 128], BF16, kind="ExternalInput")
    w3te_d = nc.dram_tensor("w3te", [128, 1], BF16, kind="ExternalInput")
    ub_d = nc.dram_tensor("ub", [1, D], BF16, kind="ExternalInput")
    # packed per-partition bias vectors: [be2 | bpr2 | b3bc | lbbc]
    bias4_d = nc.dram_tensor("bias4", [128, 4], F32, kind="ExternalInput")
    ident_d = nc.dram_tensor("ident", [128, 128], F32, kind="ExternalInput")
    out_d = nc.dram_tensor("out", [TPC, 1], F32, kind="ExternalOutput")
    dbg_d = (nc.dram_tensor("dbg", [256, 4 * NBLK], F32, kind="ExternalOutput")
             if debug_sw else None)

    with tile.TileContext(nc) as tc:
        with (
            tc.tile_pool(name="wp", bufs=1) as wp,
            tc.tile_pool(name="n1p", bufs=3) as n1p,
            tc.tile_pool(name="ep", bufs=3) as ep,
            tc.tile_pool(name="rp", bufs=3) as rp,
            tc.tile_pool(name="h0p", bufs=8) as h0p,
            tc.tile_pool(name="h1p", bufs=2) as h1p,
            tc.tile_pool(name="chp", bufs=1) as chp,
            tc.tile_pool(name="ps1p", bufs=3, space="PSUM") as ps1p,
            tc.tile_pool(name="ps2p", bufs=1, space="PSUM") as ps2p,
            tc.tile_pool(name="pssp", bufs=1, space="PSUM") as pssp,
        ):
            # ---- first block's data + layer-1 weights go FIRST so the PE
            # pipeline fills while the remaining (later-needed) constants load
            n1_0 = n1p.tile([128, 4, BLK], FP8, tag="n1")
            nc.sync.dma_start(n1_0[:], n1t_d[0])
            w1a = wp.tile([128, 4, D], FP8, tag="w1a")
            nc.sync.dma_start(
                w1a[:], w1at_d[:].rearrange("(k p) m -> p k m", p=128))
            lin1 = wp.tile([128, 4, 16], FP8, tag="lin1")
            nc.sync.dma_start(
                lin1[:], lin1t_d[:].rearrange("(k p) o -> p k o", p=128))
            ub = wp.tile([1, D], BF16, tag="ub")
            nc.sync.dma_start(ub[:], ub_d[:])
            ones = wp.tile([1, BLK], BF16, tag="ones")
            nc.vector.memset(ones[:], 1.0)
            lna = wp.tile([128, 1], F32, tag="lna")
            nc.vector.memset(lna[:], LN_A)
            # fire the exp table-set load during the weight DMAs
            warm = wp.tile([128, 1], F32, tag="warm")
            nc.scalar.activation(warm[:], lna[:], AF.Exp)
            w2 = wp.tile([128, 4 * 128], BF16, tag="w2")
            nc.sync.dma_start(
                w2[:], w2te_d[:].rearrange("(k p) m -> p k m", p=128))
            w3 = wp.tile([128, 1], BF16, tag="w3")
            nc.sync.dma_start(w3[:], w3te_d[:])
            bias4 = wp.tile([128, 4], F32, tag="bias4")
            nc.sync.dma_start(bias4[:], bias4_d[:])
            be2 = bias4[:, 0:1]
            bpr2 = bias4[:, 1:2]
            b3bc = bias4[:, 2:3]
            lbbc = bias4[:, 3:4]
            ident = wp.tile([128, 128], F32, tag="ident")
            nc.sync.dma_start(ident[:], ident_d[:])

            # s / w accumulate directly in PSUM via tokens-as-M (M=128, N=1)
            # matmuls: column 4*b+j holds tokens [b*512+j*128, ...+128).
            swAcc = pssp.tile([128, 8 * NBLK], F32, tag="swAcc")
            sAcc = swAcc[:, 0:4 * NBLK]
            wAcc = swAcc[:, 4 * NBLK:8 * NBLK]

            # ---- per-block emitters (software-pipelined below) ------------
            def emit_l1(b, n1=None):
                if n1 is None:
                    n1 = n1p.tile([128, 4, BLK], FP8, tag="n1")
                    nc.sync.dma_start(n1[:], n1t_d[b])
                h0s = []
                for pair in range(2):    # m-chunk pairs: (0,1) and (2,3)
                    ps1 = ps1p.tile([128, 2, BLK], F32, tag="ps1")
                    for mi in range(2):
                        m = 2 * pair + mi
                        for j in range(2):   # DoubleRow K pairs (K=2x128)
                            nc.tensor.matmul(
                                ps1[:, mi, :],
                                w1a[:, 2 * j:2 * j + 2,
                                    m * 128:(m + 1) * 128],
                                n1[:, 2 * j:2 * j + 2, :],
                                perf_mode=DR, start=(j == 0), stop=False)
                        # add the per-feature bias u exactly (bf16 rank-1)
                        nc.tensor.matmul(
                            ps1[:, mi, :],
                            ub[:, m * 128:(m + 1) * 128], ones[:],
                            start=False, stop=True)
                    e = ep.tile([128, 2 * BLK], BF16, tag="e")
                    nc.scalar.activation(e[:], ps1[:], AF.Exp, bias=lna[:])
                    q = rp.tile([128, 2 * BLK], BF16, tag="q")
                    nc.vector.tensor_scalar_min(q[:], e[:], A)
                    h0 = h0p.tile([128, 2 * BLK], BF16, tag="h0")
                    nc.vector.scalar_tensor_tensor(h0[:], ps1[:], 0.0, q[:],
                                                   ALU.max, ALU.add)
                    h0s.append(h0)
                for t in range(4):       # token subtiles as M
                    col = 4 * b + t
                    for j in range(2):
                        nc.tensor.matmul(
                            wAcc[:, col:col + 1],
                            n1[:, 2 * j:2 * j + 2, t * 128:(t + 1) * 128],
                            lin1[:, 2 * j:2 * j + 2, 0:1],
                            perf_mode=DR, start=(j == 0), stop=(j == 1))
                return h0s

            def emit_l2(b, h0s):
                ps2 = ps2p.tile([128, BLK], F32, tag="ps2")
                for k in range(4):
                    nc.tensor.matmul(
                        ps2[:], w2[:, k * 128:(k + 1) * 128],
                        h0s[k // 2][:, (k % 2) * BLK:(k % 2 + 1) * BLK],
                        start=(k == 0), stop=(k == 3))
                e2 = ep.tile([128, BLK], BF16, tag="e2")
                nc.scalar.activation(e2[:], ps2[:], AF.Exp, bias=be2[:])
                r2 = rp.tile([128, BLK], BF16, tag="r2")
                nc.scalar.activation(r2[:], ps2[:], AF.Relu, bias=bpr2[:])
                q2 = rp.tile([128, BLK], BF16, tag="q2")
                nc.vector.tensor_scalar_min(q2[:], e2[:], A)
                h1 = h1p.tile([128, BLK], BF16, tag="h1")
                nc.vector.tensor_tensor(h1[:], r2[:], q2[:], ALU.add)
                return h1

            def emit_l3(b, h1):
                for t in range(4):       # token subtiles as M
                    col = 4 * b + t
                    nc.tensor.matmul(sAcc[:, col:col + 1],
                                     h1[:, t * 128:(t + 1) * 128], w3[:],
                                     start=True, stop=True)

            # PE executes its queue in order: L2 of block b-1 and L3 of block
            # b-2 are emitted under L1 of block b, so the PE never waits on
            # the ACT/DVE selu chains.
            pend_l2 = None
            pend_l3 = None
            for b in range(nblk):
                h0s = emit_l1(b, n1_0 if b == 0 else None)
                if pend_l3 is not None:
                    emit_l3(*pend_l3)
                    pend_l3 = None
                if pend_l2 is not None:
                    pb, ph0s = pend_l2
                    pend_l3 = (pb, emit_l2(pb, ph0s))
                pend_l2 = (b, h0s)

            # ---- entmax_bisect (last dim of size 1) over all tokens -------
            # weight = wAcc + lin1_b;  alpha - 1 = sigmoid(weight) = 1/d
            # The w-only prefix (t1/dd/rd) depends just on wAcc, which is
            # complete after the last block's layer-1 — emit it before the
            # trailing layer-2/3 so it overlaps them instead of the tail.
            CC = 4 * NBLK
            t1 = chp.tile([128, CC], F32, tag="t1")
            nc.scalar.activation(t1[:], wAcc[:], AF.Exp,
                                 bias=lbbc[:], scale=-1.0)      # e^{-weight}
            dd = chp.tile([128, CC], F32, tag="dd")
            nc.vector.tensor_scalar_add(dd[:], t1[:], 1.0)      # 1/(alpha-1)
            rd = chp.tile([128, CC], F32, tag="rd")
            nc.vector.reciprocal(rd[:], dd[:])                  # alpha-1

            if pend_l3 is not None:
                emit_l3(*pend_l3)
            if pend_l2 is not None:
                pb, ph0s = pend_l2
                emit_l3(pb, emit_l2(pb, ph0s))

            z = chp.tile([128, CC], F32, tag="z")
            nc.vector.scalar_tensor_tensor(z[:], sAcc[:], b3bc[:], rd[:],
                                           ALU.add, ALU.mult)   # s*(alpha-1)
            tn = chp.tile([128, CC], F32, tag="tn")
            nc.vector.scalar_tensor_tensor(tn[:], z[:], 1.0, z[:],
                                           ALU.subtract, ALU.subtract)
            # tn = (z-1) - z = -(z-tau) = -t, with |t-1| <= ulp(1), so
            # ln(t) and exp(ln(t)/(alpha-1)) are bit-exact in f32 as their
            # first-order forms: ln(t) = t-1 = -tn-1, p = 1 + (t-1)*d
            # (the quadratic terms are < half-ulp for this value range).
            lq = chp.tile([128, CC], F32, tag="lq")
            nc.vector.tensor_scalar(lq[:], tn[:], -1.0, 1.0,
                                    ALU.mult, ALU.subtract)
            le = chp.tile([128, CC], F32, tag="le")
            nc.vector.tensor_tensor(le[:], lq[:], dd[:], ALU.mult)
            p = chp.tile([128, CC], F32, tag="p")
            nc.vector.tensor_scalar_add(p[:], le[:], 1.0)
            rp_ = chp.tile([128, CC], F32, tag="rp")
            nc.vector.reciprocal(rp_[:], p[:])
            res = chp.tile([128, CC], F32, tag="res")
            nc.vector.tensor_tensor(res[:], p[:], rp_[:], ALU.mult)

            # res[p, c] = token c*128 + p -> transpose so partition c holds
            # 128 contiguous tokens, then one dense store.
            rest = ps1p.tile([128, 128], F32, tag="ps1")
            nc.tensor.transpose(rest[:], res[:], ident[:])
            resT = chp.tile([128, 128], F32, tag="resT")
            nc.scalar.copy(resT[:], rest[:])
            nc.sync.dma_start(
                out_d[:].rearrange("(c p) o -> c (p o)", c=128), resT[:])
            if debug_sw:
                sdbg = chp.tile([128, CC], F32, tag="sdbg")
                nc.scalar.copy(sdbg[:], sAcc[:])
                wdbg = chp.tile([128, CC], F32, tag="wdbg")
                nc.scalar.copy(wdbg[:], wAcc[:])
                nc.sync.dma_start(dbg_d[0:128, :], sdbg[:])
                nc.sync.dma_start(dbg_d[128:256, :], wdbg[:])

    nc.compile()
    _CACHE[key] = nc
    return nc


def _prep_host(node1, u_rep, att1_w, att1_b, att2_w, att2_b, att3_w, att3_b,
               lin1_w, lin1_b):
    import ml_dtypes
    f32 = np.float32
    fp8 = ml_dtypes.float8_e4m3
    bf16 = ml_dtypes.bfloat16
    node1 = np.asarray(node1, f32)
    att1_w = np.asarray(att1_w, f32)
    att2_w = np.asarray(att2_w, f32)
    att3_w = np.asarray(att3_w, f32)
    lin1_w = np.asarray(lin1_w, f32)
    u_rep = np.asarray(u_rep, f32)
    C = np.float32(SC * A)

    # layer 1: u_rep's contribution + att1_b as per-feature bias u
    u_bias = (att1_w[:, D:] @ u_rep[0] + np.asarray(att1_b, f32)).astype(f32)
    w1at = np.ascontiguousarray(att1_w[:, :D].T).astype(fp8)   # [D, D]
    ub = np.ascontiguousarray(u_bias.reshape(1, D)).astype(bf16)

    # selu affine (selu = SC*nc - SC*A) folded into layer 2
    w2te = np.ascontiguousarray((SC * att2_w.T).astype(bf16))  # [D, 128]
    b2_eff = (np.asarray(att2_b, f32) - C * att2_w.sum(axis=1)).astype(f32)
    be2 = (b2_eff + np.float32(LN_A)).reshape(128, 1)
    bpr2 = b2_eff.reshape(128, 1).copy()

    # selu affine folded into layer 3
    w3te = np.ascontiguousarray((SC * att3_w.T).astype(bf16))  # [128, 1]
    b3_eff = np.float32(np.asarray(att3_b, f32)[0] - C * att3_w.sum())

    lin1t = np.zeros((D, 16), f32)
    lin1t[:, 0] = lin1_w[0]
    lin1t = lin1t.astype(fp8)                                  # [D, 16] padded
    b3bc = np.full((128, 1), b3_eff, f32)
    lbbc = np.full((128, 1), -np.float32(np.asarray(lin1_b, f32)[0]), f32)
    ident = np.eye(128, dtype=f32)

    bias4 = np.ascontiguousarray(
        np.concatenate([be2, bpr2, b3bc, lbbc], axis=1))
    shared = dict(w1at=w1at, lin1t=lin1t, ub=ub, w2te=w2te, w3te=w3te,
                  bias4=bias4, ident=ident)
    in_maps = []
    for c in range(N_CORES):
        m = dict(shared)
        nt = np.ascontiguousarray(
            node1[c * TPC:(c + 1) * TPC, :].T).astype(fp8)
        # [D, TPC] -> block-major [NBLK, 128, 4, BLK] with
        # [b, p, k, t] = nt[k*128 + p, b*BLK + t]
        m["n1t"] = np.ascontiguousarray(
            nt.reshape(4, 128, NBLK, BLK).transpose(2, 1, 0, 3)
        ).reshape(NBLK, 128, 4 * BLK)
        in_maps.append(m)
    return in_maps


def kernel(node1, u_rep, att1_w, att1_b, att2_w, att2_b, att3_w, att3_b,
           lin1_w, lin1_b, num_neighs=None, **_unused):
    nc = _build()
    in_maps = _prep_host(node1, u_rep, att1_w, att1_b, att2_w, att2_b,
                         att3_w, att3_b, lin1_w, lin1_b)
    res = run_bass_kernel_spmd(nc, in_maps, core_ids=list(range(N_CORES)))
    out = np.concatenate([res.results[c]["out"] for c in range(N_CORES)],
                         axis=0)
    return out.astype(np.float32)


# revision 53
# speedup vs baseline: 8272.7579x; 1.0020x over previous
"""Trainium2 Bass kernel for nn_Attention_79998060855419 (sparse_attention).

Reference pipeline per row i of node1 [131072, 512]:
    x      = concat(node1[i], u_rep)                     # [1024]
    weight = node1[i] @ lin1_w.T + lin1_b                # [1]
    alpha  = sigmoid(weight) + 1                         # in (1, 2)
    h0     = selu(x @ att1_w.T + att1_b)                 # [512]
    h1     = selu(h0 @ att2_w.T + att2_b)                # [128]
    s      = h1 @ att3_w.T + att3_b                      # [1]
    out[i] = entmax_bisect(s, alpha)  over dim of size 1 # [1]

Distribution: pure data-parallel over the neighbor axis — 8 cores x 16384
rows; the tiny MLP weights and u_rep are replicated (per the sharding hint).
No collectives are needed; each core computes its shard's output.

Device-side dataflow (per core, 32 blocks of 512 tokens):
  - Activations flow transposed (features on partitions, tokens on the free
    axis): node1 is fed as node1.T tiles and the layer matmuls are
    weights-stationary.  The row-reductions (lin1, att3) run tokens-as-M
    (M=128, N=1) and accumulate straight into a persistent PSUM tile, so
    the entmax stage is one [128, 128] pass with no staging copies.
  - Host prep only reshapes/transposes inputs and folds biases and the selu
    affine constants into downstream weights — all FLOPs over node1-derived
    data run on the NeuronCores.
  - Layer 1 (512x512) and lin1 run on the TensorEngine in fp8(e4m3) with
    perf_mode=DoubleRow (contraction packed in K-pairs, FD=512); layers 2/3
    run in bf16.  The final entmax normalization makes the output invariant
    to these precision choices (p/p == 1.0 bit-exactly either way).
  - selu(t): the per-feature bias u enters the PSUM through an exact bf16
    rank-1 (u x ones) K=1 matmul in the same accumulation group, so t sits
    in PSUM and every selu scalar is a constant; layer-1 m-chunks are
    processed as [128, 2, 512] two-bank PSUM pairs:
        e' = exp(t + ln A)              (ScalarE, PSUM -> SBUF bf16)
        q  = min(e', A)                 (VectorE tensor_scalar, bf16 4x)
        nc = max(t, 0) + q              (VectorE scalar_tensor_tensor)
    which equals selu(t)/SC + A; the affine map selu = SC*nc - SC*A is
    folded into the next layer's weights/bias on the host.
  - entmax_bisect with last-dim size 1 degenerates: tau_hi == tau_lo == z-1
    and dm0 == 0, so all 50 bisection iterations compute
    p = clip(z - (z-1), 0)^(1/(alpha-1)) and return p / sum(p) = p / p.
    The kernel computes exactly that: z = s*(alpha-1), t = z - (z-1)
    (so |t-1| <= ulp(1)), p = t^(1/(alpha-1)) evaluated via the
    first-order forms ln(t) = t-1 and exp(x) = 1+x — bit-exact in f32 for
    this value range since the quadratic terms sit below half-ulp — and
    out = p * recip(p).  The result is exactly 1.0 for every finite
    positive p, on device and in the reference alike.
"""

import math

import numpy as np

import concourse.bacc as bacc
import concourse.mybir as mybir
import concourse.tile as tile
from concourse.bass_utils import run_bass_kernel_spmd

N = 131072
D = 512
N_CORES = 8
TPC = N // N_CORES          # tokens per core = 16384
BLK = 512                   # tokens per block
NBLK = TPC // BLK           # 32 blocks per core
NROW = NBLK

SC = 1.0507009873554804934193349852946   # selu scale
A = 1.6732632423543772848170429916717    # selu alpha
LN_A = math.log(A)

F32 = mybir.dt.float32
FP8 = mybir.dt.float8e4      # e4m3
BF16 = mybir.dt.bfloat16
AF = mybir.ActivationFunctionType
ALU = mybir.AluOpType
DR = mybir.MatmulPerfMode.DoubleRow

_CACHE = {}


def _build(nblk=NBLK, debug_sw=False):
    key = ("nc", nblk, debug_sw)
    if key in _CACHE:
        return _CACHE[key]

    nc = bacc.Bacc("TRN2", target_bir_lowering=False, debug=False,
                   num_devices=N_CORES)

    # Per-core inputs (shard of node1.T + replicated, host-folded weights).
    # block-major node1.T: [block, partition, k-chunk * tokens], so each
    # block load is one fully contiguous 2KB-per-partition DMA
    n1t_d = nc.dram_tensor("n1t", [NBLK, 128, 4 * BLK], FP8,
                           kind="ExternalInput")
    w1at_d = nc.dram_tensor("w1at", [D, D], FP8, kind="ExternalInput")
    lin1t_d = nc.dram_tensor("lin1t", [D, 16], FP8, kind="ExternalInput")
    w2te_d = nc.dram_tensor("w2te", [D, 128], BF16, kind="ExternalInput")
    w3te_d = nc.dram_tensor("w3te", [128, 1], BF16, kind="ExternalInput")
    ub_d = nc.dram_tensor("ub", [1, D], BF16, kind="ExternalInput")
    # packed per-partition bias vectors: [be2 | bpr2 | b3bc | lbbc]
    bias4_d = nc.dram_tensor("bias4", [128, 4], F32, kind="ExternalInput")
    ident_d = nc.dram_tensor("ident", [128, 128], F32, kind="ExternalInput")
    out_d = nc.dram_tensor("out", [TPC, 1], F32, kind="ExternalOutput")
    dbg_d = (nc.dram_tensor("dbg", [256, 4 * NBLK], F32, kind="ExternalOutput")
             if debug_sw else None)

    with tile.TileContext(nc) as tc:
        with (
            tc.tile_pool(name="wp", bufs=1) as wp,
            tc.tile_pool(name="n1p", bufs=3) as n1p,
            tc.tile_pool(name="ep", bufs=3) as ep,
            tc.tile_pool(name="rp", bufs=3) as rp,
            tc.tile_pool(name="h0p", bufs=8) as h0p,
            tc.tile_pool(name="h1p", bufs=2) as h1p,
            tc.tile_pool(name="chp", bufs=1) as chp,
            tc.tile_pool(name="ps1p", bufs=3, space="PSUM") as ps1p,
            tc.tile_pool(name="ps2p", bufs=1, space="PSUM") as ps2p,
            tc.tile_pool(name="pssp", bufs=1, space="PSUM") as pssp,
        ):
            # ---- first block's data + layer-1 weights go FIRST so the PE
            # pipeline fills while the remaining (later-needed) constants load
            n1_0 = n1p.tile([128, 4, BLK], FP8, tag="n1")
            nc.sync.dma_start(n1_0[:], n1t_d[0])
            w1a = wp.tile([128, 4, D], FP8, tag="w1a")
            nc.sync.dma_start(
                w1a[:], w1at_d[:].rearrange("(k p) m -> p k m", p=128))
            lin1 = wp.tile([128, 4, 16], FP8, tag="lin1")
            nc.sync.dma_start(
                lin1[:], lin1t_d[:].rearrange("(k p) o -> p k o", p=128))
            ub = wp.tile([1, D], BF16, tag="ub")
            nc.sync.dma_start(ub[:], ub_d[:])
            # prefetch blocks 1-2 ahead of the later-needed constants so the
            # early steady-state never waits on the DMA queue
            n1_1 = n1p.tile([128, 4, BLK], FP8, tag="n1")
            nc.sync.dma_start(n1_1[:], n1t_d[1])
            n1_2 = n1p.tile([128, 4, BLK], FP8, tag="n1")
            nc.sync.dma_start(n1_2[:], n1t_d[2])
            ones = wp.tile([1, BLK], BF16, tag="ones")
            nc.vector.memset(ones[:], 1.0)
            lna = wp.tile([128, 1], F32, tag="lna")
            nc.vector.memset(lna[:], LN_A)
            # fire the exp table-set load during the weight DMAs
            warm = wp.tile([128, 1], F32, tag="warm")
            nc.scalar.activation(warm[:], lna[:], AF.Exp)
            w2 = wp.tile([128, 4 * 128], BF16, tag="w2")
            nc.sync.dma_start(
                w2[:], w2te_d[:].rearrange("(k p) m -> p k m", p=128))
            w3 = wp.tile([128, 1], BF16, tag="w3")
            nc.sync.dma_start(w3[:], w3te_d[:])
            bias4 = wp.tile([128, 4], F32, tag="bias4")
            nc.sync.dma_start(bias4[:], bias4_d[:])
            be2 = bias4[:, 0:1]
            bpr2 = bias4[:, 1:2]
            b3bc = bias4[:, 2:3]
            lbbc = bias4[:, 3:4]
            ident = wp.tile([128, 128], F32, tag="ident")

            # s / w accumulate directly in PSUM via tokens-as-M (M=128, N=1)
            # matmuls: column 4*b+j holds tokens [b*512+j*128, ...+128).
            swAcc = pssp.tile([128, 8 * NBLK], F32, tag="swAcc")
            sAcc = swAcc[:, 0:4 * NBLK]
            wAcc = swAcc[:, 4 * NBLK:8 * NBLK]

            # ---- per-block emitters (software-pipelined below) ------------
            def emit_l1(b, n1=None):
                if n1 is None:
                    n1 = n1p.tile([128, 4, BLK], FP8, tag="n1")
                    nc.sync.dma_start(n1[:], n1t_d[b])
                h0s = []
                for pair in range(2):    # m-chunk pairs: (0,1) and (2,3)
                    ps1 = ps1p.tile([128, 2, BLK], F32, tag="ps1")
                    for mi in range(2):
                        m = 2 * pair + mi
                        for j in range(2):   # DoubleRow K pairs (K=2x128)
                            nc.tensor.matmul(
                                ps1[:, mi, :],
                                w1a[:, 2 * j:2 * j + 2,
                                    m * 128:(m + 1) * 128],
                                n1[:, 2 * j:2 * j + 2, :],
                                perf_mode=DR, start=(j == 0), stop=False)
                        # add the per-feature bias u exactly (bf16 rank-1)
                        nc.tensor.matmul(
                            ps1[:, mi, :],
                            ub[:, m * 128:(m + 1) * 128], ones[:],
                            start=False, stop=True)
                    e = ep.tile([128, 2 * BLK], BF16, tag="e")
                    nc.scalar.activation(e[:], ps1[:], AF.Exp, bias=lna[:])
                    q = rp.tile([128, 2 * BLK], BF16, tag="q")
                    nc.vector.tensor_scalar_min(q[:], e[:], A)
                    h0 = h0p.tile([128, 2 * BLK], BF16, tag="h0")
                    nc.vector.scalar_tensor_tensor(h0[:], ps1[:], 0.0, q[:],
                                                   ALU.max, ALU.add)
                    h0s.append(h0)
                for t in range(4):       # token subtiles as M
                    col = 4 * b + t
                    for j in range(2):
                        nc.tensor.matmul(
                            wAcc[:, col:col + 1],
                            n1[:, 2 * j:2 * j + 2, t * 128:(t + 1) * 128],
                            lin1[:, 2 * j:2 * j + 2, 0:1],
                            perf_mode=DR, start=(j == 0), stop=(j == 1))
                return h0s

            def emit_l2(b, h0s):
                ps2 = ps2p.tile([128, BLK], F32, tag="ps2")
                for k in range(4):
                    nc.tensor.matmul(
                        ps2[:], w2[:, k * 128:(k + 1) * 128],
                        h0s[k // 2][:, (k % 2) * BLK:(k % 2 + 1) * BLK],
                        start=(k == 0), stop=(k == 3))
                e2 = ep.tile([128, BLK], BF16, tag="e2")
                nc.scalar.activation(e2[:], ps2[:], AF.Exp, bias=be2[:])
                r2 = rp.tile([128, BLK], BF16, tag="r2")
                nc.scalar.activation(r2[:], ps2[:], AF.Relu, bias=bpr2[:])
                q2 = rp.tile([128, BLK], BF16, tag="q2")
                nc.vector.tensor_scalar_min(q2[:], e2[:], A)
                h1 = h1p.tile([128, BLK], BF16, tag="h1")
                nc.vector.tensor_tensor(h1[:], r2[:], q2[:], ALU.add)
                return h1

            def emit_l3(b, h1):
                for t in range(4):       # token subtiles as M
                    col = 4 * b + t
                    nc.tensor.matmul(sAcc[:, col:col + 1],
                                     h1[:, t * 128:(t + 1) * 128], w3[:],
                                     start=True, stop=True)

            # PE executes its queue in order: L2 of block b-1 and L3 of block
            # b-2 are emitted under L1 of block b, so the PE never waits on
            # the ACT/DVE selu chains.
            pend_l2 = None
            pend_l3 = None
            pre = {0: n1_0, 1: n1_1, 2: n1_2}
            for b in range(nblk):
                h0s = emit_l1(b, pre.get(b))
                if pend_l3 is not None:
                    emit_l3(*pend_l3)
                    pend_l3 = None
                if pend_l2 is not None:
                    pb, ph0s = pend_l2
                    pend_l3 = (pb, emit_l2(pb, ph0s))
                pend_l2 = (b, h0s)

            # ---- entmax_bisect (last dim of size 1) over all tokens -------
            # weight = wAcc + lin1_b;  alpha - 1 = sigmoid(weight) = 1/d
            # The w-only prefix (t1/dd/rd) depends just on wAcc, which is
            # complete after the last block's layer-1 — emit it before the
            # trailing layer-2/3 so it overlaps them instead of the tail.
            CC = 4 * NBLK
            t1 = chp.tile([128, CC], F32, tag="t1")
            nc.scalar.activation(t1[:], wAcc[:], AF.Exp,
                                 bias=lbbc[:], scale=-1.0)      # e^{-weight}
            dd = chp.tile([128, CC], F32, tag="dd")
            nc.vector.tensor_scalar_add(dd[:], t1[:], 1.0)      # 1/(alpha-1)
            rd = chp.tile([128, CC], F32, tag="rd")
            nc.vector.reciprocal(rd[:], dd[:])                  # alpha-1

            if pend_l3 is not None:
                emit_l3(*pend_l3)
            if pend_l2 is not None:
                pb, ph0s = pend_l2
                emit_l3(pb, emit_l2(pb, ph0s))
            # identity for the final transpose — needed only now
            nc.sync.dma_start(ident[:], ident_d[:])

            z = chp.tile([128, CC], F32, tag="z")
            nc.vector.scalar_tensor_tensor(z[:], sAcc[:], b3bc[:], rd[:],
                                           ALU.add, ALU.mult)   # s*(alpha-1)
            tn = chp.tile([128, CC], F32, tag="tn")
            nc.vector.scalar_tensor_tensor(tn[:], z[:], 1.0, z[:],
                                           ALU.subtract, ALU.subtract)
            # tn = (z-1) - z = -(z-tau) = -t, with |t-1| <= ulp(1), so
            # ln(t) and exp(ln(t)/(alpha-1)) are bit-exact in f32 as their
            # first-order forms: ln(t) = t-1 = -tn-1, p = 1 + (t-1)*d
            # (the quadratic terms are < half-ulp for this value range).
            lq = chp.tile([128, CC], F32, tag="lq")
            nc.vector.tensor_scalar(lq[:], tn[:], -1.0, 1.0,
                                    ALU.mult, ALU.subtract)
            le = chp.tile([128, CC], F32, tag="le")
            nc.vector.tensor_tensor(le[:], lq[:], dd[:], ALU.mult)
            p = chp.tile([128, CC], F32, tag="p")
            nc.vector.tensor_scalar_add(p[:], le[:], 1.0)
            rp_ = chp.tile([128, CC], F32, tag="rp")
            nc.vector.reciprocal(rp_[:], p[:])
            res = chp.tile([128, CC], F32, tag="res")
            nc.vector.tensor_tensor(res[:], p[:], rp_[:], ALU.mult)

            # res[p, c] = token c*128 + p -> transpose so partition c holds
            # 128 contiguous tokens, then one dense store.
            rest = ps1p.tile([128, 128], F32, tag="ps1")
            nc.tensor.transpose(rest[:], res[:], ident[:])
            resT = chp.tile([128, 128], F32, tag="resT")
            nc.scalar.copy(resT[:], rest[:])
            nc.sync.dma_start(
                out_d[:].rearrange("(c p) o -> c (p o)", c=128), resT[:])
            if debug_sw:
                sdbg = chp.tile([128, CC], F32, tag="sdbg")
                nc.scalar.copy(sdbg[:], sAcc[:])
                wdbg = chp.tile([128, CC], F32, tag="wdbg")
                nc.scalar.copy(wdbg[:], wAcc[:])
                nc.sync.dma_start(dbg_d[0:128, :], sdbg[:])
                nc.sync.dma_start(dbg_d[128:256, :], wdbg[:])

    nc.compile()
    _CACHE[key] = nc
    return nc


def _prep_host(node1, u_rep, att1_w, att1_b, att2_w, att2_b, att3_w, att3_b,
               lin1_w, lin1_b):
    import ml_dtypes
    f32 = np.float32
    fp8 = ml_dtypes.float8_e4m3
    bf16 = ml_dtypes.bfloat16
    node1 = np.asarray(node1, f32)
    att1_w = np.asarray(att1_w, f32)
    att2_w = np.asarray(att2_w, f32)
    att3_w = np.asarray(att3_w, f32)
    lin1_w = np.asarray(lin1_w, f32)
    u_rep = np.asarray(u_rep, f32)
    C = np.float32(SC * A)

    # layer 1: u_rep's contribution + att1_b as per-feature bias u
    u_bias = (att1_w[:, D:] @ u_rep[0] + np.asarray(att1_b, f32)).astype(f32)
    w1at = np.ascontiguousarray(att1_w[:, :D].T).astype(fp8)   # [D, D]
    ub = np.ascontiguousarray(u_bias.reshape(1, D)).astype(bf16)

    # selu affine (selu = SC*nc - SC*A) folded into layer 2
    w2te = np.ascontiguousarray((SC * att2_w.T).astype(bf16))  # [D, 128]
    b2_eff = (np.asarray(att2_b, f32) - C * att2_w.sum(axis=1)).astype(f32)
    be2 = (b2_eff + np.float32(LN_A)).reshape(128, 1)
    bpr2 = b2_eff.reshape(128, 1).copy()

    # selu affine folded into layer 3
    w3te = np.ascontiguousarray((SC * att3_w.T).astype(bf16))  # [128, 1]
    b3_eff = np.float32(np.asarray(att3_b, f32)[0] - C * att3_w.sum())

    lin1t = np.zeros((D, 16), f32)
    lin1t[:, 0] = lin1_w[0]
    lin1t = lin1t.astype(fp8)                                  # [D, 16] padded
    b3bc = np.full((128, 1), b3_eff, f32)
    lbbc = np.full((128, 1), -np.float32(np.asarray(lin1_b, f32)[0]), f32)
    ident = np.eye(128, dtype=f32)

    bias4 = np.ascontiguousarray(
        np.concatenate([be2, bpr2, b3bc, lbbc], axis=1))
    shared = dict(w1at=w1at, lin1t=lin1t, ub=ub, w2te=w2te, w3te=w3te,
                  bias4=bias4, ident=ident)
    in_maps = []
    for c in range(N_CORES):
        m = dict(shared)
        nt = np.ascontiguousarray(
            node1[c * TPC:(c + 1) * TPC, :].T).astype(fp8)
        # [D, TPC] -> block-major [NBLK, 128, 4, BLK] with
        # [b, p, k, t] = nt[k*128 + p, b*BLK + t]
        m["n1t"] = np.ascontiguousarray(
            nt.reshape(4, 128, NBLK, BLK).transpose(2, 1, 0, 3)
        ).reshape(NBLK, 128, 4 * BLK)
        in_maps.append(m)
    return in_maps


def kernel(node1, u_rep, att1_w, att1_b, att2_w, att2_b, att3_w, att3_b,
           lin1_w, lin1_b, num_neighs=None, **_unused):
    nc = _build()
    in_maps = _prep_host(node1, u_rep, att1_w, att1_b, att2_w, att2_b,
                         att3_w, att3_b, lin1_w, lin1_b)
    res = run_bass_kernel_spmd(nc, in_maps, core_ids=list(range(N_CORES)))
    out = np.concatenate([res.results[c]["out"] for c in range(N_CORES)],
                         axis=0)
    return out.astype(np.float32)


# revision 56
# speedup vs baseline: 8286.7262x; 1.0017x over previous
"""Trainium2 Bass kernel for nn_Attention_79998060855419 (sparse_attention).

Reference pipeline per row i of node1 [131072, 512]:
    x      = concat(node1[i], u_rep)                     # [1024]
    weight = node1[i] @ lin1_w.T + lin1_b                # [1]
    alpha  = sigmoid(weight) + 1                         # in (1, 2)
    h0     = selu(x @ att1_w.T + att1_b)                 # [512]
    h1     = selu(h0 @ att2_w.T + att2_b)                # [128]
    s      = h1 @ att3_w.T + att3_b                      # [1]
    out[i] = entmax_bisect(s, alpha)  over dim of size 1 # [1]

Distribution: pure data-parallel over the neighbor axis — 8 cores x 16384
rows; the tiny MLP weights and u_rep are replicated (per the sharding hint).
No collectives are needed; each core computes its shard's output.

Device-side dataflow (per core, 32 blocks of 512 tokens):
  - Activations flow transposed (features on partitions, tokens on the free
    axis): node1 is fed as node1.T tiles and the layer matmuls are
    weights-stationary.  The row-reductions (lin1, att3) run tokens-as-M
    (M=128, N=1) and accumulate straight into a persistent PSUM tile, so
    the entmax stage is one [128, 128] pass with no staging copies.
  - Host prep only reshapes/transposes inputs and folds biases and the selu
    affine constants into downstream weights — all FLOPs over node1-derived
    data run on the NeuronCores.
  - Layer 1 (512x512) and lin1 run on the TensorEngine in fp8(e4m3) with
    perf_mode=DoubleRow (contraction packed in K-pairs, FD=512); layers 2/3
    run in bf16.  The final entmax normalization makes the output invariant
    to these precision choices (p/p == 1.0 bit-exactly either way).
  - selu(t): the per-feature bias u enters the PSUM through an exact bf16
    rank-1 (u x ones) K=1 matmul in the same accumulation group, so t sits
    in PSUM and every selu scalar is a constant; layer-1 m-chunks are
    processed as [128, 2, 512] two-bank PSUM pairs:
        e' = exp(t + ln A)              (ScalarE, PSUM -> SBUF bf16)
        q  = min(e', A)                 (VectorE tensor_scalar, bf16 4x)
        nc = max(t, 0) + q              (VectorE scalar_tensor_tensor)
    which equals selu(t)/SC + A; the affine map selu = SC*nc - SC*A is
    folded into the next layer's weights/bias on the host.
  - entmax_bisect with last-dim size 1 degenerates: tau_hi == tau_lo == z-1
    and dm0 == 0, so all 50 bisection iterations compute
    p = clip(z - (z-1), 0)^(1/(alpha-1)) and return p / sum(p) = p / p.
    The kernel computes exactly that: z = s*(alpha-1), t = z - (z-1)
    (so |t-1| <= ulp(1)), p = t^(1/(alpha-1)) evaluated via the
    first-order forms ln(t) = t-1 and exp(x) = 1+x — bit-exact in f32 for
    this value range since the quadratic terms sit below half-ulp — and
    out = p * recip(p).  The result is exactly 1.0 for every finite
    positive p, on device and in the reference alike.
"""

import math

import numpy as np

import concourse.bacc as bacc
import concourse.mybir as mybir
import concourse.tile as tile
from concourse.bass_utils import run_bass_kernel_spmd

N = 131072
D = 512
N_CORES = 8
TPC = N // N_CORES          # tokens per core = 16384
BLK = 512                   # tokens per block
NBLK = TPC // BLK           # 32 blocks per core
NROW = NBLK

SC = 1.0507009873554804934193349852946   # selu scale
A = 1.6732632423543772848170429916717    # selu alpha
LN_A = math.log(A)

F32 = mybir.dt.float32
FP8 = mybir.dt.float8e4      # e4m3
BF16 = mybir.dt.bfloat16
AF = mybir.ActivationFunctionType
ALU = mybir.AluOpType
DR = mybir.MatmulPerfMode.DoubleRow

_CACHE = {}


def _build(nblk=NBLK, debug_sw=False):
    key = ("nc", nblk, debug_sw)
    if key in _CACHE:
        return _CACHE[key]

    nc = bacc.Bacc("TRN2", target_bir_lowering=False, debug=False,
                   num_devices=N_CORES)

    # Per-core inputs (shard of node1.T + replicated, host-folded weights).
    # block-major node1.T: [block, partition, k-chunk * tokens], so each
    # block load is one fully contiguous 2KB-per-partition DMA
    n1t_d = nc.dram_tensor("n1t", [NBLK, 128, 4 * BLK], FP8,
                           kind="ExternalInput")
    w1at_d = nc.dram_tensor("w1at", [D, D], FP8, kind="ExternalInput")
    lin1t_d = nc.dram_tensor("lin1t", [D, 16], FP8, kind="ExternalInput")
    w2te_d = nc.dram_tensor("w2te", [D, 128], BF16, kind="ExternalInput")
    w3te_d = nc.dram_tensor("w3te", [128, 1], BF16, kind="ExternalInput")
    ub_d = nc.dram_tensor("ub", [1, D], BF16, kind="ExternalInput")
    # packed per-partition bias vectors: [be2 | bpr2 | b3bc | lbbc]
    bias4_d = nc.dram_tensor("bias4", [128, 4], F32, kind="ExternalInput")
    ident_d = nc.dram_tensor("ident", [128, 128], F32, kind="ExternalInput")
    out_d = nc.dram_tensor("out", [TPC, 1], F32, kind="ExternalOutput")
    dbg_d = (nc.dram_tensor("dbg", [256, 4 * NBLK], F32, kind="ExternalOutput")
             if debug_sw else None)

    with tile.TileContext(nc) as tc:
        with (
            tc.tile_pool(name="wp", bufs=1) as wp,
            tc.tile_pool(name="n1p", bufs=3) as n1p,
            tc.tile_pool(name="ep", bufs=3) as ep,
            tc.tile_pool(name="rp", bufs=3) as rp,
            tc.tile_pool(name="h0p", bufs=8) as h0p,
            tc.tile_pool(name="h1p", bufs=2) as h1p,
            tc.tile_pool(name="chp", bufs=1) as chp,
            tc.tile_pool(name="ps1p", bufs=3, space="PSUM") as ps1p,
            tc.tile_pool(name="ps2p", bufs=1, space="PSUM") as ps2p,
            tc.tile_pool(name="pssp", bufs=1, space="PSUM") as pssp,
        ):
            # ---- first block's data + layer-1 weights go FIRST so the PE
            # pipeline fills while the remaining (later-needed) constants load
            n1_0 = n1p.tile([128, 4, BLK], FP8, tag="n1")
            nc.sync.dma_start(n1_0[:], n1t_d[0])
            w1a = wp.tile([128, 4, D], FP8, tag="w1a")
            nc.sync.dma_start(
                w1a[:], w1at_d[:].rearrange("(k p) m -> p k m", p=128))
            lin1 = wp.tile([128, 4, 16], FP8, tag="lin1")
            nc.sync.dma_start(
                lin1[:], lin1t_d[:].rearrange("(k p) o -> p k o", p=128))
            ub = wp.tile([1, D], BF16, tag="ub")
            nc.sync.dma_start(ub[:], ub_d[:])
            # prefetch blocks 1-2 ahead of the later-needed constants so the
            # early steady-state never waits on the DMA queue
            n1_1 = n1p.tile([128, 4, BLK], FP8, tag="n1")
            nc.sync.dma_start(n1_1[:], n1t_d[1])
            n1_2 = n1p.tile([128, 4, BLK], FP8, tag="n1")
            nc.sync.dma_start(n1_2[:], n1t_d[2])
            ones = wp.tile([1, BLK], BF16, tag="ones")
            nc.vector.memset(ones[:], 1.0)
            lna = wp.tile([128, 1], F32, tag="lna")
            nc.vector.memset(lna[:], LN_A)
            # fire the exp table-set load during the weight DMAs
            warm = wp.tile([128, 1], F32, tag="warm")
            nc.scalar.activation(warm[:], lna[:], AF.Exp)
            w2 = wp.tile([128, 4 * 128], BF16, tag="w2")
            nc.sync.dma_start(
                w2[:], w2te_d[:].rearrange("(k p) m -> p k m", p=128))
            w3 = wp.tile([128, 1], BF16, tag="w3")
            nc.sync.dma_start(w3[:], w3te_d[:])
            bias4 = wp.tile([128, 4], F32, tag="bias4")
            nc.sync.dma_start(bias4[:], bias4_d[:])
            be2 = bias4[:, 0:1]
            bpr2 = bias4[:, 1:2]
            b3bc = bias4[:, 2:3]
            lbbc = bias4[:, 3:4]
            ident = wp.tile([128, 128], F32, tag="ident")

            # s / w accumulate directly in PSUM via tokens-as-M (M=128, N=1)
            # matmuls: column 4*b+j holds tokens [b*512+j*128, ...+128).
            swAcc = pssp.tile([128, 8 * NBLK], F32, tag="swAcc")
            sAcc = swAcc[:, 0:4 * NBLK]
            wAcc = swAcc[:, 4 * NBLK:8 * NBLK]

            # ---- per-block emitters (software-pipelined below) ------------
            def emit_l1(b, n1=None):
                if n1 is None:
                    n1 = n1p.tile([128, 4, BLK], FP8, tag="n1")
                    nc.sync.dma_start(n1[:], n1t_d[b])
                h0s = []
                for pair in range(2):    # m-chunk pairs: (0,1) and (2,3)
                    ps1 = ps1p.tile([128, 2, BLK], F32, tag="ps1")
                    for mi in range(2):
                        m = 2 * pair + mi
                        for j in range(2):   # DoubleRow K pairs (K=2x128)
                            nc.tensor.matmul(
                                ps1[:, mi, :],
                                w1a[:, 2 * j:2 * j + 2,
                                    m * 128:(m + 1) * 128],
                                n1[:, 2 * j:2 * j + 2, :],
                                perf_mode=DR, start=(j == 0), stop=False)
                        # add the per-feature bias u exactly (bf16 rank-1)
                        nc.tensor.matmul(
                            ps1[:, mi, :],
                            ub[:, m * 128:(m + 1) * 128], ones[:],
                            start=False, stop=True)
                    e = ep.tile([128, 2 * BLK], BF16, tag="e")
                    nc.scalar.activation(e[:], ps1[:], AF.Exp, bias=lna[:])
                    q = rp.tile([128, 2 * BLK], BF16, tag="q")
                    nc.vector.tensor_scalar_min(q[:], e[:], A)
                    h0 = h0p.tile([128, 2 * BLK], BF16, tag="h0")
                    nc.vector.scalar_tensor_tensor(h0[:], ps1[:], 0.0, q[:],
                                                   ALU.max, ALU.add)
                    h0s.append(h0)
                for t in range(4):       # token subtiles as M
                    col = 4 * b + t
                    for j in range(2):
                        nc.tensor.matmul(
                            wAcc[:, col:col + 1],
                            n1[:, 2 * j:2 * j + 2, t * 128:(t + 1) * 128],
                            lin1[:, 2 * j:2 * j + 2, 0:1],
                            perf_mode=DR, start=(j == 0), stop=(j == 1))
                return h0s

            def emit_l2(b, h0s):
                ps2 = ps2p.tile([128, BLK], F32, tag="ps2")
                for k in range(4):
                    nc.tensor.matmul(
                        ps2[:], w2[:, k * 128:(k + 1) * 128],
                        h0s[k // 2][:, (k % 2) * BLK:(k % 2 + 1) * BLK],
                        start=(k == 0), stop=(k == 3))
                e2 = ep.tile([128, BLK], BF16, tag="e2")
                nc.scalar.activation(e2[:], ps2[:], AF.Exp, bias=be2[:])
                r2 = rp.tile([128, BLK], BF16, tag="r2")
                nc.scalar.activation(r2[:], ps2[:], AF.Relu, bias=bpr2[:])
                q2 = rp.tile([128, BLK], BF16, tag="q2")
                nc.vector.tensor_scalar_min(q2[:], e2[:], A)
                h1 = h1p.tile([128, BLK], BF16, tag="h1")
                nc.vector.tensor_tensor(h1[:], r2[:], q2[:], ALU.add)
                return h1

            def emit_l3(b, h1):
                for t in range(4):       # token subtiles as M
                    col = 4 * b + t
                    nc.tensor.matmul(sAcc[:, col:col + 1],
                                     h1[:, t * 128:(t + 1) * 128], w3[:],
                                     start=True, stop=True)

            # PE executes its queue in order: L2 of block b-1 and L3 of block
            # b-2 are emitted under L1 of block b, so the PE never waits on
            # the ACT/DVE selu chains.
            pend_l2 = None
            pend_l3 = None
            pre = {0: n1_0, 1: n1_1, 2: n1_2}
            for b in range(nblk):
                h0s = emit_l1(b, pre.get(b))
                if pend_l3 is not None:
                    emit_l3(*pend_l3)
                    pend_l3 = None
                if pend_l2 is not None:
                    pb, ph0s = pend_l2
                    pend_l3 = (pb, emit_l2(pb, ph0s))
                pend_l2 = (b, h0s)

            # ---- entmax_bisect (last dim of size 1) over all tokens -------
            # weight = wAcc + lin1_b;  alpha - 1 = sigmoid(weight) = 1/d
            # The w-only prefix (t1/dd/rd) depends just on wAcc, which is
            # complete after the last block's layer-1 — emit it before the
            # trailing layer-2/3 so it overlaps them instead of the tail.
            CC = 4 * NBLK
            t1 = chp.tile([128, CC], F32, tag="t1")
            nc.scalar.activation(t1[:], wAcc[:], AF.Exp,
                                 bias=lbbc[:], scale=-1.0)      # e^{-weight}
            dd = chp.tile([128, CC], F32, tag="dd")
            nc.vector.tensor_scalar_add(dd[:], t1[:], 1.0)      # 1/(alpha-1)
            rd = chp.tile([128, CC], F32, tag="rd")
            nc.vector.reciprocal(rd[:], dd[:])                  # alpha-1

            if pend_l3 is not None:
                emit_l3(*pend_l3)
            if pend_l2 is not None:
                pb, ph0s = pend_l2
                emit_l3(pb, emit_l2(pb, ph0s))
            # identity for the final transpose — needed only now
            nc.sync.dma_start(ident[:], ident_d[:])

            z = chp.tile([128, CC], F32, tag="z")
            nc.vector.scalar_tensor_tensor(z[:], sAcc[:], b3bc[:], rd[:],
                                           ALU.add, ALU.mult)   # s*(alpha-1)
            tn = chp.tile([128, CC], F32, tag="tn")
            nc.vector.scalar_tensor_tensor(tn[:], z[:], 1.0, z[:],
                                           ALU.subtract, ALU.subtract)
            # tn = (z-1) - z = -(z-tau) = -t, with |t-1| <= ulp(1), so
            # ln(t) and exp(ln(t)/(alpha-1)) are bit-exact in f32 as their
            # first-order forms: ln(t) = t-1 = -tn-1, p = 1 + (t-1)*d
            # (the quadratic terms are < half-ulp for this value range).
            nle = chp.tile([128, CC], F32, tag="nle")
            nc.vector.scalar_tensor_tensor(nle[:], tn[:], 1.0, dd[:],
                                           ALU.add, ALU.mult)
            # nle = (tn+1)*d = -(t-1)*d;  p = 1 - nle = 1 + (t-1)*d
            p = chp.tile([128, CC], F32, tag="p")
            nc.vector.tensor_scalar(p[:], nle[:], -1.0, 1.0,
                                    ALU.mult, ALU.add)
            rp_ = chp.tile([128, CC], F32, tag="rp")
            nc.vector.reciprocal(rp_[:], p[:])
            res = chp.tile([128, CC], F32, tag="res")
            nc.vector.tensor_tensor(res[:], p[:], rp_[:], ALU.mult)

            # res[p, c] = token c*128 + p -> transpose so partition c holds
            # 128 contiguous tokens, then one dense store.
            rest = ps1p.tile([128, 128], F32, tag="ps1")
            nc.tensor.transpose(rest[:], res[:], ident[:])
            resT = chp.tile([128, 128], F32, tag="resT")
            nc.scalar.copy(resT[:], rest[:])
            nc.sync.dma_start(
                out_d[:].rearrange("(c p) o -> c (p o)", c=128), resT[:])
            if debug_sw:
                sdbg = chp.tile([128, CC], F32, tag="sdbg")
                nc.scalar.copy(sdbg[:], sAcc[:])
                wdbg = chp.tile([128, CC], F32, tag="wdbg")
                nc.scalar.copy(wdbg[:], wAcc[:])
                nc.sync.dma_start(dbg_d[0:128, :], sdbg[:])
                nc.sync.dma_start(dbg_d[128:256, :], wdbg[:])

    nc.compile()
    _CACHE[key] = nc
    return nc


def _prep_host(node1, u_rep, att1_w, att1_b, att2_w, att2_b, att3_w, att3_b,
               lin1_w, lin1_b):
    import ml_dtypes
    f32 = np.float32
    fp8 = ml_dtypes.float8_e4m3
    bf16 = ml_dtypes.bfloat16
    node1 = np.asarray(node1, f32)
    att1_w = np.asarray(att1_w, f32)
    att2_w = np.asarray(att2_w, f32)
    att3_w = np.asarray(att3_w, f32)
    lin1_w = np.asarray(lin1_w, f32)
    u_rep = np.asarray(u_rep, f32)
    C = np.float32(SC * A)

    # layer 1: u_rep's contribution + att1_b as per-feature bias u
    u_bias = (att1_w[:, D:] @ u_rep[0] + np.asarray(att1_b, f32)).astype(f32)
    w1at = np.ascontiguousarray(att1_w[:, :D].T).astype(fp8)   # [D, D]
    ub = np.ascontiguousarray(u_bias.reshape(1, D)).astype(bf16)

    # selu affine (selu = SC*nc - SC*A) folded into layer 2
    w2te = np.ascontiguousarray((SC * att2_w.T).astype(bf16))  # [D, 128]
    b2_eff = (np.asarray(att2_b, f32) - C * att2_w.sum(axis=1)).astype(f32)
    be2 = (b2_eff + np.float32(LN_A)).reshape(128, 1)
    bpr2 = b2_eff.reshape(128, 1).copy()

    # selu affine folded into layer 3
    w3te = np.ascontiguousarray((SC * att3_w.T).astype(bf16))  # [128, 1]
    b3_eff = np.float32(np.asarray(att3_b, f32)[0] - C * att3_w.sum())

    lin1t = np.zeros((D, 16), f32)
    lin1t[:, 0] = lin1_w[0]
    lin1t = lin1t.astype(fp8)                                  # [D, 16] padded
    b3bc = np.full((128, 1), b3_eff, f32)
    lbbc = np.full((128, 1), -np.float32(np.asarray(lin1_b, f32)[0]), f32)
    ident = np.eye(128, dtype=f32)

    bias4 = np.ascontiguousarray(
        np.concatenate([be2, bpr2, b3bc, lbbc], axis=1))
    shared = dict(w1at=w1at, lin1t=lin1t, ub=ub, w2te=w2te, w3te=w3te,
                  bias4=bias4, ident=ident)
    in_maps = []
    for c in range(N_CORES):
        m = dict(shared)
        nt = np.ascontiguousarray(
            node1[c * TPC:(c + 1) * TPC, :].T).astype(fp8)
        # [D, TPC] -> block-major [NBLK, 128, 4, BLK] with
        # [b, p, k, t] = nt[k*128 + p, b*BLK + t]
        m["n1t"] = np.ascontiguousarray(
            nt.reshape(4, 128, NBLK, BLK).transpose(2, 1, 0, 3)
        ).reshape(NBLK, 128, 4 * BLK)
        in_maps.append(m)
    return in_maps


def kernel(node1, u_rep, att1_w, att1_b, att2_w, att2_b, att3_w, att3_b,
           lin1_w, lin1_b, num_neighs=None, **_unused):
    nc = _build()
    in_maps = _prep_host(node1, u_rep, att1_w, att1_b, att2_w, att2_b,
                         att3_w, att3_b, lin1_w, lin1_b)
    res = run_bass_kernel_spmd(nc, in_maps, core_ids=list(range(N_CORES)))
    out = np.concatenate([res.results[c]["out"] for c in range(N_CORES)],
                         axis=0)
    return out.astype(np.float32)
